# revision 34
# baseline (speedup 1.0000x reference)
"""Trainium2 Bass kernel for nn_Autotuner_FFN (dense MLP, 8-core data parallel).

Fast-path structure (be1=be2=0, bc2=0 — true for this model's inputs):
  * Host precomputes the feature matrix XT [256, B] in fp16: one-hot
    encodings, the 57 sign(x)*ln(|x|+1) transformed features, a ones row
    carrying the folded first-layer bias, zero padding to 2 full K=128
    tiles. LayerNorm affine g is folded into W1/W2 columns (stats use a
    per-partition 1/g prescale); mean-centering is folded into weights.
  * Per 512-sample chunk the device does only:
      L1: 16 fp16 matmuls -> G1 (PSUM)
      LN1: Act Square(G1)->fp8 pairs, DVE relu(G1)->f16 R1 (pv1 DEFERRED)
      stats1: 4 fp8 DoubleRow ones-reduce matmuls; pv1 = AbsRsqrt LUT
      L2: 64 fp16 matmuls over unnormalized R1 -> G2
      LN2: same; variance rescaled by pv1^2 in [1,512] smalls
      L3: 8 fp16 matmuls -> g3; y = pv1*pv2*g3 (+b3)
    LayerNorm scale-invariance makes the deferral exact: relu(c*x) =
    c*relu(x) for c>0, so per-column scales commute out to the end.
  * No PSUM->SBUF copies, no rsqrt broadcast matmuls, no bias adds, no
    device transcendentals except the one AbsRsqrt LUT per LN.
  * All matmul K-tiles are padded to 128 (K<128 matmuls run ~2x slower).
Legacy general path (arbitrary affine/bias) retained below.
"""
import numpy as np

import concourse.bass as bass
import concourse.tile as tile
from concourse import bacc, mybir
from concourse.bass_utils import run_bass_kernel_spmd

AF = mybir.ActivationFunctionType
ALU = mybir.AluOpType
F32 = mybir.dt.float32
F16 = mybir.dt.float16
F8 = mybir.dt.float8e4
DRM = mybir.MatmulPerfMode.DoubleRow
F32R = mybir.dt.float16  # legacy alias

B = 65536
N_CORES = 8
B_CORE = B // N_CORES          # 8192
CH = 512                       # batch chunk (one PSUM bank wide)
NCH = B_CORE // CH             # 16
HID = 1024
MT = HID // 128                # 8 hidden m-tiles
KA, KC = 128, 57               # legacy feature K tiles
EPS = 1e-5
LN2 = float(np.log(2.0))


# ---------------------------------------------------------------- host folds
def _fold_weights(inp):
    f8 = lambda x: np.asarray(x, np.float64)
    W1 = f8(inp["W1"]); b1 = f8(inp["b1"])
    emb_kc = f8(inp["emb_kc"]); emb_nl = f8(inp["emb_nl"])
    op_W = f8(inp["op_W"]); op_b = f8(inp["op_b"])
    emb_c = f8(inp["emb_contig"]); emb_s = f8(inp["emb_scalar"])
    emb_i = f8(inp["emb_indirect"])
    H = W1.shape[1]
    rows_A = []
    bias = b1.copy()
    rows_A.append(emb_kc @ W1[0:16])
    rows_A.append(emb_nl @ W1[16:32])
    W1_op = W1[32:944].reshape(57, 16, H)
    rows_A.append(np.einsum("ij,ijh->ih", op_W, W1_op))
    bias += np.einsum("ij,ijh->h", op_b, W1_op)
    rd_f2, rd_bool, rd_ss = [], [], []
    wd_f2, wd_bool, wd_ss = [], [], []
    for base, f2l, booll, ssl in ((947, rd_f2, rd_bool, rd_ss),
                                  (1027, wd_f2, wd_bool, wd_ss)):
        for d in range(4):
            Wd = W1[base + 20 * d: base + 20 * d + 20]
            f2l.append(Wd[0:2])
            ssl.append(Wd[2:8] / LN2)
            rows_b = []
            for e, sl in ((emb_c, slice(8, 12)), (emb_s, slice(12, 16)),
                          (emb_i, slice(16, 20))):
                rows_b.append((e[1] - e[0]) @ Wd[sl])
                bias += e[0] @ Wd[sl]
            booll.append(np.stack(rows_b))
    rows_A += [np.concatenate(rd_f2), np.concatenate(rd_bool),
               np.concatenate(wd_f2), np.concatenate(wd_bool),
               W1[1110:1112]]
    A = np.concatenate(rows_A)
    C = np.concatenate([W1[944:947] / LN2, W1[1107:1110] / LN2,
                        W1[1112:1115] / LN2,
                        np.concatenate(rd_ss), np.concatenate(wd_ss)])
    W1_eff = np.concatenate([A, np.zeros((3, H)), C])       # [185, H]
    W1c = W1_eff - W1_eff.mean(axis=1, keepdims=True)
    bc1 = bias - bias.mean()
    W2 = f8(inp["W2"]); b2 = f8(inp["b2"])
    W2c = W2 - W2.mean(axis=1, keepdims=True)
    bc2 = b2 - b2.mean()
    return (W1c.astype(np.float32), bc1.astype(np.float32),
            W2c.astype(np.float32), bc2.astype(np.float32))


def _build_xt_raw(inp):
    """[185, B] float32 feature matrix, 57 transform rows still raw."""
    Bn = inp["op_vec"].shape[0]
    kc = np.asarray(inp["kernel_category_idx"]).astype(np.int64)
    nl = np.asarray(inp["num_of_loops_idx"]).astype(np.int64)
    f = lambda k: np.asarray(inp[k], np.float32)
    XT = np.zeros((185, Bn), np.float32)
    XT[0:10] = (np.arange(10)[:, None] == kc[None, :])
    XT[10:26] = (np.arange(16)[:, None] == nl[None, :])
    XT[26:83] = f("op_vec").T
    XT[83:91] = f("read_dep_float")[:, :, 0:2].reshape(Bn, 8).T
    XT[91:103] = np.asarray(inp["read_dep_bools"]).reshape(Bn, 12).T
    XT[103:111] = f("write_dep_float")[:, :, 0:2].reshape(Bn, 8).T
    XT[111:123] = np.asarray(inp["write_dep_bools"]).reshape(Bn, 12).T
    XT[123:125] = f("rest_vec")[:, 3:5].T
    XT[128:131] = f("size_hints").T
    XT[131:137] = f("rest_vec")[:, [0, 1, 2, 5, 6, 7]].T
    XT[137:161] = f("read_dep_float")[:, :, 2:8].reshape(Bn, 24).T
    XT[161:185] = f("write_dep_float")[:, :, 2:8].reshape(Bn, 24).T
    return XT


def _pack128(v):
    """[1024] -> [128, 8] with v[m*128+p] at [p, m]."""
    return np.ascontiguousarray(np.asarray(v, np.float32).reshape(8, 128).T)


# ---------------------------------------------------------------- fast2 device
# fp8 hi+lo split-matmul path. All matmuls (except L3) run as fp8e4m3
# DoubleRow 3-pass Karatsuba: A@B ~ Ah@Bh + Al@Bh + Ah@Bl, with operands
# pre-scaled by powers of two so every fp8 value sits in the normal range
# (the naive split fails because W values ~0.03 put the lo term in
# subnormals). Scale domains: X*SX, W1*SW1 -> G1 psum = D1*g1*h1;
# W2*SW2 -> G2 psum = D2*g2*h2'. relu-hi/lo extraction then needs no
# scaling at all: hi = fp8(max(G,0)), lo = fp8(max(G,0)-hi).
SX, SW1, SW2 = 1.0, 16.0, 32.0
D1 = SX * SW1
D2 = D1 * SW2

FAST2_CFG = dict(xin_bufs=3, rh_bufs=2, rl_bufs=2, sq_bufs=2, r2_bufs=2,
                 sm_bufs=3, ps_mm_bufs=3, ps_st_bufs=1, ps_p3_bufs=1,
                 skew=True, skew_depth=1)


def build_fast2(scal, loop_iters=None, cfg=None):
    """scal = (a1, a2, qd, b3f): Square scales for LN1/LN2, final dequant
    pv1*pv2*qd, final bias (baked)."""
    a1, a2, qd, b3f = scal
    cfg = {**FAST2_CFG, **(cfg or {})}
    nc = bacc.Bacc("TRN2", target_bir_lowering=False, debug=False)
    xh = nc.dram_tensor("xh", [128, 2 * B_CORE], F8, kind="ExternalInput")
    xl = nc.dram_tensor("xl", [128, 2 * B_CORE], F8, kind="ExternalInput")
    w1h = nc.dram_tensor("w1h", [128, 2 * HID], F8, kind="ExternalInput")
    w1l = nc.dram_tensor("w1l", [128, 2 * HID], F8, kind="ExternalInput")
    w2h = nc.dram_tensor("w2h", [128, 8 * HID], F8, kind="ExternalInput")
    w2l = nc.dram_tensor("w2l", [128, 8 * HID], F8, kind="ExternalInput")
    w3p = nc.dram_tensor("w3p", [128, MT], F16, kind="ExternalInput")
    y = nc.dram_tensor("y", [1, B_CORE], F32, kind="ExternalOutput")
    KP = 4                                      # hidden 128-row pair tiles

    from contextlib import ExitStack
    with tile.TileContext(nc) as tc, ExitStack() as ctx, \
            nc.allow_low_precision(reason="fp8 hi/lo split is intentional"):
        const = ctx.enter_context(tc.tile_pool(name="const", bufs=1))
        xin = ctx.enter_context(tc.tile_pool(name="xin", bufs=cfg["xin_bufs"]))
        rhp = ctx.enter_context(tc.tile_pool(name="rhp", bufs=cfg["rh_bufs"]))
        rlp = ctx.enter_context(tc.tile_pool(name="rlp", bufs=cfg["rl_bufs"]))
        sqp = ctx.enter_context(tc.tile_pool(name="sqp", bufs=cfg["sq_bufs"]))
        r2p = ctx.enter_context(tc.tile_pool(name="r2p", bufs=cfg["r2_bufs"]))
        sm = ctx.enter_context(tc.tile_pool(name="sm", bufs=cfg["sm_bufs"]))
        ps_mm = ctx.enter_context(
            tc.tile_pool(name="ps_mm", bufs=cfg["ps_mm_bufs"], space="PSUM"))
        ps_st = ctx.enter_context(
            tc.tile_pool(name="ps_st", bufs=cfg["ps_st_bufs"], space="PSUM"))
        ps_p3 = ctx.enter_context(
            tc.tile_pool(name="ps_p3", bufs=cfg["ps_p3_bufs"], space="PSUM"))

        def load_const(name, dram, shape, dt):
            t = const.tile(shape, dt, tag=name)
            nc.sync.dma_start(t[:], dram.ap())
            return t
        w1h_t = load_const("w1h_t", w1h, [128, 2 * HID], F8)
        w1l_t = load_const("w1l_t", w1l, [128, 2 * HID], F8)
        w2h_t = load_const("w2h_t", w2h, [128, 8 * HID], F8)
        w2l_t = load_const("w2l_t", w2l, [128, 8 * HID], F8)
        w3r = load_const("w3r", w3p, [128, MT], F16)
        ones_st = const.tile([128, 2, 32], F32, tag="ones_st")
        nc.vector.memset(ones_st[:], 1.0)
        ones8 = const.tile([128, 2, 32], F8, tag="ones8")
        nc.vector.tensor_copy(ones8[:], ones_st[:])
        eps_t = const.tile([1, 1], F32, tag="eps_t")
        nc.vector.memset(eps_t[:], EPS)

        w1h3 = w1h_t[:].rearrange("p (j n) -> p j n", j=2)
        w1l3 = w1l_t[:].rearrange("p (j n) -> p j n", j=2)
        w2h4 = w2h_t[:].rearrange("p (k j n) -> p k j n", k=KP, j=2)
        w2l4 = w2l_t[:].rearrange("p (k j n) -> p k j n", k=KP, j=2)
        xh3 = xh.ap().rearrange("p (j b) -> p j b", j=2)
        xl3 = xl.ap().rearrange("p (j b) -> p j b", j=2)

        def stats_block(sqs, tag):
            stw = ps_st.tile([32, CH], F32, name=f"stw{tag}", tag="stw")
            for i, sq in enumerate(sqs):
                nc.tensor.matmul(stw[:], ones8[:],
                                 sq[:].rearrange("p (j c) -> p j c", j=2),
                                 start=(i == 0), stop=(i == len(sqs) - 1),
                                 perf_mode=DRM)
            return stw

        def chunk_partA(cs):
            """DMA x hi/lo, L1 3-pass DRM matmuls, LN1 elementwise."""
            xh_t = xin.tile([128, 2, CH], F8, name="xh_t", tag="xh_t")
            nc.sync.dma_start(xh_t[:], xh3[:, :, cs])
            xl_t = xin.tile([128, 2, CH], F8, name="xl_t", tag="xl_t")
            nc.sync.dma_start(xl_t[:], xl3[:, :, cs])
            his, los, sqs = [], [], []
            for kp in range(KP):
                p = ps_mm.tile([128, 2 * CH], F32, name="pmm", tag="pmm")
                for m2 in range(2):
                    sl = slice((2 * kp + m2) * 128, (2 * kp + m2 + 1) * 128)
                    out = p[:, m2 * CH:(m2 + 1) * CH]
                    nc.tensor.matmul(out, w1h3[:, :, sl], xh_t[:],
                                     start=True, stop=False, perf_mode=DRM)
                    nc.tensor.matmul(out, w1l3[:, :, sl], xh_t[:],
                                     start=False, stop=False, perf_mode=DRM)
                    nc.tensor.matmul(out, w1h3[:, :, sl], xl_t[:],
                                     start=False, stop=True, perf_mode=DRM)
                sq = sqp.tile([128, 2 * CH], F8, name=f"sq1_{kp}",
                              tag=f"sq1_{kp}")
                nc.scalar.activation(sq[:], p[:], AF.Square, scale=a1)
                hi = rhp.tile([128, 2 * CH], F8, name=f"hi1_{kp}",
                              tag=f"hi1_{kp}")
                nc.vector.tensor_scalar(out=hi[:], in0=p[:], scalar1=0.0,
                                        scalar2=None, op0=ALU.max)
                lo = rlp.tile([128, 2 * CH], F8, name=f"lo1_{kp}",
                              tag=f"lo1_{kp}")
                nc.vector.scalar_tensor_tensor(
                    out=lo[:], in0=p[:], scalar=0.0, in1=hi[:],
                    op0=ALU.max, op1=ALU.subtract)
                his.append(hi); los.append(lo); sqs.append(sq)
            return his, los, sqs

        def chunk_partB(cs, his, los, sqs1):
            st1 = stats_block(sqs1, "1")
            pv1 = sm.tile([1, CH], F32, name="pv1", tag="pv1")
            nc.scalar.activation(pv1[:], st1[0:1, :], AF.Abs_reciprocal_sqrt,
                                 bias=eps_t[:], scale=1.0 / HID)

            hi3 = [h[:].rearrange("p (j c) -> p j c", j=2) for h in his]
            lo3 = [l[:].rearrange("p (j c) -> p j c", j=2) for l in los]
            r2s, sqs2 = [], []
            for kp in range(KP):
                p = ps_mm.tile([128, 2 * CH], F32, name="pmm", tag="pmm")
                for m2 in range(2):
                    sl = slice((2 * kp + m2) * 128, (2 * kp + m2 + 1) * 128)
                    out = p[:, m2 * CH:(m2 + 1) * CH]
                    nk = 3 * KP
                    i = 0
                    for k in range(KP):
                        nc.tensor.matmul(out, w2h4[:, k, :, sl], hi3[k],
                                         start=(i == 0), stop=(i == nk - 1),
                                         perf_mode=DRM); i += 1
                    for k in range(KP):
                        nc.tensor.matmul(out, w2l4[:, k, :, sl], hi3[k],
                                         start=False, stop=(i == nk - 1),
                                         perf_mode=DRM); i += 1
                    for k in range(KP):
                        nc.tensor.matmul(out, w2h4[:, k, :, sl], lo3[k],
                                         start=False, stop=(i == nk - 1),
                                         perf_mode=DRM); i += 1
                sq = sqp.tile([128, 2 * CH], F8, name=f"sq2_{kp}",
                              tag=f"sq2_{kp}")
                nc.scalar.activation(sq[:], p[:], AF.Square, scale=a2)
                r2 = r2p.tile([128, 2 * CH], F16, name=f"r2_{kp}",
                              tag=f"r2_{kp}")
                nc.scalar.activation(r2[:], p[:], AF.Relu)
                r2s.append(r2); sqs2.append(sq)

            st2 = stats_block(sqs2, "2")
            t1 = sm.tile([1, CH], F32, name="t1", tag="t1")
            nc.vector.tensor_mul(t1[:], pv1[:], pv1[:])
            u1 = sm.tile([1, CH], F32, name="u1", tag="u1")
            nc.vector.tensor_mul(u1[:], t1[:], st2[0:1, :])
            pv2 = sm.tile([1, CH], F32, name="pv2", tag="pv2")
            nc.scalar.activation(pv2[:], u1[:], AF.Abs_reciprocal_sqrt,
                                 bias=eps_t[:], scale=1.0 / HID)
            q2 = sm.tile([1, CH], F32, name="q2", tag="q2")
            nc.vector.scalar_tensor_tensor(
                out=q2[:], in0=pv1[:], scalar=qd, in1=pv2[:],
                op0=ALU.mult, op1=ALU.mult)

            p3 = ps_p3.tile([1, CH], F32, name="p3", tag="p3")
            for k in range(MT):
                nc.tensor.matmul(p3[:], w3r[:, k:k + 1],
                                 r2s[k // 2][:, (k % 2) * CH:(k % 2 + 1) * CH],
                                 start=(k == 0), stop=(k == MT - 1))
            osb = sm.tile([1, CH], F32, name="osb", tag="osb")
            nc.vector.tensor_mul(osb[:], p3[:], q2[:])
            if b3f != 0.0:
                nc.vector.tensor_scalar(out=osb[:], in0=osb[:], scalar1=b3f,
                                        scalar2=None, op0=ALU.add)
            nc.sync.dma_start(y.ap()[0:1, cs], osb[:])

        def _cs(c):
            return slice(c * CH, (c + 1) * CH)

        def whole_body():
            if cfg.get("skew", True):
                depth = cfg.get("skew_depth", 1)
                pend = [chunk_partA(_cs(c)) for c in range(min(depth, NCH))]
                for c in range(NCH):
                    if c + depth < NCH:
                        pend.append(chunk_partA(_cs(c + depth)))
                    chunk_partB(_cs(c), *pend.pop(0))
            else:
                for c in range(NCH):
                    chunk_partB(_cs(c), *chunk_partA(_cs(c)))

        if loop_iters is None:
            whole_body()
        else:
            with tc.For_i(0, loop_iters, 1):
                whole_body()
    nc.compile()
    return nc


def _pack_rows(W, groups):
    """[groups*128, N] -> [128, groups*N] with row g*128+p at [p, g*N:...]"""
    W = np.asarray(W)
    n = W.shape[1]
    return np.ascontiguousarray(
        W.reshape(groups, 128, n).transpose(1, 0, 2).reshape(128, groups * n))


def _q8(x):
    import ml_dtypes
    return np.asarray(x, np.float32).astype(ml_dtypes.float8_e4m3)


def make_fast2_maps(inp):
    """Host prep for the fp8 split path. Returns (in_maps, scal)."""
    W1c, bc1, W2c, bc2 = _fold_weights(inp)
    g1 = np.asarray(inp["g1"], np.float64)
    g2 = np.asarray(inp["g2"], np.float64)
    W3 = np.asarray(inp["W3"], np.float32)
    b3 = np.asarray(inp["b3"], np.float32)

    XT = _build_xt_raw(inp)
    Xc = XT[128:185]
    XT[128:185] = np.sign(Xc) * np.log(np.abs(Xc) + 1.0)
    XTF = np.zeros((256, XT.shape[1]), np.float32)
    XTF[0:185] = XT
    XTF[185] = 1.0

    W1g = np.zeros((256, HID))
    W1g[0:185] = W1c.astype(np.float64) * g1[None, :]
    W1g[185] = bc1.astype(np.float64) * g1
    W2g = W2c.astype(np.float64) * g2[None, :]

    xs = (XTF * SX).astype(np.float32)
    xh8 = _q8(xs)
    xl8 = _q8(xs - xh8.astype(np.float32))
    w1s = (W1g * SW1).astype(np.float32)
    w1h8 = _q8(w1s)
    w1l8 = _q8(w1s - w1h8.astype(np.float32))
    w2s = (W2g * SW2).astype(np.float32)
    w2h8 = _q8(w2s)
    w2l8 = _q8(w2s - w2h8.astype(np.float32))

    g1u = float(g1[0])
    g2u = float(g2[0])
    scal = (1.0 / (D1 * g1u), 1.0 / (D2 * g2u), 1.0 / D2, float(b3[0]))
    shared = {
        "w1h": _pack_rows(w1h8, 2), "w1l": _pack_rows(w1l8, 2),
        "w2h": _pack_rows(w2h8, 8), "w2l": _pack_rows(w2l8, 8),
        "w3p": _pack128(W3[:, 0]).astype(np.float16),
    }
    xh8p = _pack_rows(xh8, 2).reshape(128, 2, B)
    xl8p = _pack_rows(xl8, 2).reshape(128, 2, B)
    in_maps = []
    for c in range(N_CORES):
        m = dict(shared)
        sl = slice(c * B_CORE, (c + 1) * B_CORE)
        m["xh"] = np.ascontiguousarray(xh8p[:, :, sl]).reshape(128, 2 * B_CORE)
        m["xl"] = np.ascontiguousarray(xl8p[:, :, sl]).reshape(128, 2 * B_CORE)
        in_maps.append(m)
    return in_maps, scal


def fast2_ok(inp):
    be1 = np.asarray(inp["be1"]); be2 = np.asarray(inp["be2"])
    g1 = np.asarray(inp["g1"]); g2 = np.asarray(inp["g2"])
    _, _, _, bc2 = _fold_weights(inp)
    return (np.all(be1 == 0.0) and np.all(be2 == 0.0)
            and np.all(np.abs(bc2) < 1e-12)
            and np.all(g1 == g1[0]) and np.all(g2 == g2[0])
            and abs(g1[0]) > 1e-6 and abs(g2[0]) > 1e-6)


# ---------------------------------------------------------------- fast3 device
# fp16 matmuls (the empirical PE cost is ~226ns/instruction regardless of
# dtype/K, so fp8 hi-lo splitting loses: it needs 1.5x the instructions)
# with pair-PSUM tiles [128, 2*CH] and pair-wide elementwise ops: half the
# Act/DVE instructions and semaphore traffic of the per-m-tile layout.
FAST3_CFG = dict(xin_bufs=3, r_bufs=2, sq_bufs=2, sm_bufs=3,
                 ps_mm_bufs=3, ps_st_bufs=1, ps_p3_bufs=1,
                 skew=True, skew_depth=1, mm_interleave=1)


def build_fast3(scal, loop_iters=None, cfg=None):
    """scal = (a1, a2, qd, b3f) baked scalar scales (uniform g)."""
    a1, a2, qd, b3f = scal
    cfg = {**FAST3_CFG, **(cfg or {})}
    unified = cfg.get("unified_psum", False)
    nc = bacc.Bacc("TRN2", target_bir_lowering=False, debug=False)
    xt = nc.dram_tensor("xt", [128, 2 * B_CORE], F16, kind="ExternalInput")
    w1 = nc.dram_tensor("w1", [256, HID], F16, kind="ExternalInput")
    w2 = nc.dram_tensor("w2", [HID, HID], F16, kind="ExternalInput")
    w3p = nc.dram_tensor("w3p", [128, MT], F16, kind="ExternalInput")
    y = nc.dram_tensor("y", [1, B_CORE], F32, kind="ExternalOutput")
    KP = 4

    from contextlib import ExitStack
    with tile.TileContext(nc) as tc, ExitStack() as ctx, \
            nc.allow_low_precision(reason="fp16/fp8 rounding is intentional"):
        const = ctx.enter_context(tc.tile_pool(name="const", bufs=1))
        xin = ctx.enter_context(tc.tile_pool(name="xin", bufs=cfg["xin_bufs"]))
        rp = ctx.enter_context(tc.tile_pool(name="rp", bufs=cfg["r_bufs"]))
        sqp = ctx.enter_context(tc.tile_pool(name="sqp", bufs=cfg["sq_bufs"]))
        sm = ctx.enter_context(tc.tile_pool(name="sm", bufs=cfg["sm_bufs"]))
        ps_mm = ctx.enter_context(
            tc.tile_pool(name="ps_mm",
                         bufs=(4 if unified else cfg["ps_mm_bufs"]),
                         space="PSUM"))
        if unified:
            ps_st = ps_p3 = ps_mm
        else:
            ps_st = ctx.enter_context(
                tc.tile_pool(name="ps_st", bufs=cfg["ps_st_bufs"],
                             space="PSUM"))
            ps_p3 = ctx.enter_context(
                tc.tile_pool(name="ps_p3", bufs=cfg["ps_p3_bufs"],
                             space="PSUM"))

        w1a = const.tile([128, HID], F16, tag="w1a")
        nc.sync.dma_start(w1a[:], w1.ap()[0:128, :])
        w1b = const.tile([128, HID], F16, tag="w1b")
        nc.sync.dma_start(w1b[:], w1.ap()[128:256, :])
        w2r = []
        for k in range(MT):
            t = const.tile([128, HID], F16, name=f"w2r{k}", tag=f"w2r{k}")
            nc.sync.dma_start(t[:], w2.ap()[k * 128:(k + 1) * 128, :])
            w2r.append(t)
        w3r = const.tile([128, MT], F16, tag="w3r")
        nc.sync.dma_start(w3r[:], w3p.ap())
        ones_st = const.tile([128, 2, 32], F32, tag="ones_st")
        nc.vector.memset(ones_st[:], 1.0)
        ones8 = const.tile([128, 2, 32], F8, tag="ones8")
        nc.vector.tensor_copy(ones8[:], ones_st[:])
        eps_t = const.tile([1, 1], F32, tag="eps_t")
        nc.vector.memset(eps_t[:], EPS)
        xt3 = xt.ap().rearrange("p (j b) -> p j b", j=2)

        def stats_block(sqs, tag):
            if unified:
                stw_t = ps_mm.tile([128, 2 * CH], F32, name=f"stw{tag}",
                                   tag="pmm")
                stw = stw_t[0:32, 0:CH]
            else:
                stw = ps_st.tile([32, CH], F32, name=f"stw{tag}",
                                 tag="stw")[:]
            for i, sq in enumerate(sqs):
                nc.tensor.matmul(stw, ones8[:],
                                 sq[:].rearrange("p (j c) -> p j c", j=2),
                                 start=(i == 0), stop=(i == len(sqs) - 1),
                                 perf_mode=DRM)
            return stw

        def emit_relu(r_ap, p_ap, kp):
            """Relu PSUM->f16, engine chosen per pair by cfg."""
            from contextlib import nullcontext
            mode = cfg.get("relu_eng", "dve")
            eng = mode if mode in ("dve", "act", "half") else \
                ("act" if kp % 2 else "dve")
            hp = cfg.get("hp_relu", 0)
            with (tc.high_priority(offset=hp) if hp else nullcontext()):
                if eng == "half":
                    nc.vector.tensor_scalar(out=r_ap[:, 0:CH],
                                            in0=p_ap[:, 0:CH], scalar1=0.0,
                                            scalar2=None, op0=ALU.max)
                    nc.scalar.activation(r_ap[:, CH:2 * CH],
                                         p_ap[:, CH:2 * CH], AF.Relu)
                elif eng == "act":
                    nc.scalar.activation(r_ap, p_ap, AF.Relu)
                else:
                    nc.vector.tensor_scalar(out=r_ap, in0=p_ap, scalar1=0.0,
                                            scalar2=None, op0=ALU.max)

        def layer_pairs(w_list, rhs_list, sq_scale, out_tag, out_dt):
            """KP pair-psums, each 2 regions x len(w_list) fp16 chains;
            pair-wide Square->fp8 and Relu->out_dt. Returns (R pair tiles,
            sq pair tiles)."""
            il = cfg.get("mm_interleave", 1)
            rs, sqs = [], []
            for kp0 in range(0, KP, il):
                kps = list(range(kp0, min(kp0 + il, KP)))
                ptiles = [ps_mm.tile([128, 2 * CH], F32, name=f"pm{kp}",
                                     tag="pmm") for kp in kps]
                nk = len(w_list)
                for k in range(nk):
                    for m2 in range(2):
                        for p, kp in zip(ptiles, kps):
                            m = 2 * kp + m2
                            nc.tensor.matmul(
                                p[:, m2 * CH:(m2 + 1) * CH],
                                w_list[k][:, m * 128:(m + 1) * 128],
                                rhs_list[k], start=(k == 0),
                                stop=(k == nk - 1))
                for p, kp in zip(ptiles, kps):
                    sq = sqp.tile([128, 2 * CH], F8, name=f"{out_tag}sq{kp}",
                                  tag=f"{out_tag}sq{kp}")
                    nc.scalar.activation(sq[:], p[:], AF.Square,
                                         scale=sq_scale)
                    r = rp.tile([128, 2 * CH], out_dt, name=f"{out_tag}{kp}",
                                tag=f"{out_tag}{kp}")
                    emit_relu(r[:], p[:], kp)
                    rs.append(r); sqs.append(sq)
            return rs, sqs

        def l1_pair(x_t, kp):
            """One L1 pair-psum: 4 fp16 matmuls + sq + relu."""
            p = ps_mm.tile([128, 2 * CH], F32, name=f"pa{kp}", tag="pmm")
            for k in range(2):
                for m2 in range(2):
                    m = 2 * kp + m2
                    nc.tensor.matmul(p[:, m2 * CH:(m2 + 1) * CH],
                                     (w1a, w1b)[k][:, m * 128:(m + 1) * 128],
                                     x_t[:, k, :], start=(k == 0),
                                     stop=(k == 1))
            sq = sqp.tile([128, 2 * CH], F8, name=f"R1sq{kp}",
                          tag=f"R1sq{kp}")
            if cfg.get("swap_eng", False):
                nc.vector.tensor_mul(sq[:], p[:], p[:])
                r = rp.tile([128, 2 * CH], F16, name=f"R1{kp}",
                            tag=f"R1{kp}")
                nc.scalar.activation(r[:], p[:], AF.Relu)
            else:
                nc.scalar.activation(sq[:], p[:], AF.Square, scale=a1)
                r = rp.tile([128, 2 * CH], F16, name=f"R1{kp}",
                            tag=f"R1{kp}")
                emit_relu(r[:], p[:], kp)
            if cfg.get("dma_launder", False):
                rd = rp.tile([128, 2 * CH], F16, name=f"R1d{kp}",
                             tag=f"R1d{kp}")
                nc.sync.dma_start(rd[:], r[:])
                return rd, sq
            return r, sq

        def chunk_partA(cs):
            x_t = xin.tile([128, 2, CH], F16, name="x_t", tag="x_t")
            nc.sync.dma_start(x_t[:], xt3[:, :, cs])
            R1, sq1 = [], []
            for kp in range(KP):
                r, sq = l1_pair(x_t, kp)
                R1.append(r); sq1.append(sq)
            return R1, sq1

        def chunk_partA_dma(cs):
            x_t = xin.tile([128, 2, CH], F16, name="x_t", tag="x_t")
            nc.sync.dma_start(x_t[:], xt3[:, :, cs])
            return x_t

        # diagnostic: constant rhs tiles to cut matmul->elementwise deps
        if cfg.get("dep_cut", False):
            cst = const.tile([128, 2 * CH], F16, tag="cst")
            nc.vector.memset(cst[:], 0.01)
            cst8 = const.tile([128, 2 * CH], F8, tag="cst8")
            nc.vector.memset(cst8[:], 0.01)

        def chunk_partB(cs, R1, sq1, next_x=None, next_out=None):
            if cfg.get("dep_cut", False):
                R1 = [cst] * KP
                sq1 = [cst8] * KP
            st1 = stats_block(sq1, "1")
            pv1 = sm.tile([1, CH], F32, name="pv1", tag="pv1")
            nc.scalar.activation(pv1[:], st1[0:1, :], AF.Abs_reciprocal_sqrt,
                                 bias=eps_t[:], scale=1.0 / HID)

            r1sl = [R1[k // 2][:, (k % 2) * CH:(k % 2 + 1) * CH]
                    for k in range(MT)]
            if next_x is None:
                R2, sq2 = layer_pairs(w2r, r1sl, a2, "R2", F16)
            else:
                # software-pipeline: slot next chunk's L1 pairs between this
                # chunk's L2 pairs so PE never bursts ahead of Act/DVE.
                R2, sq2 = [], []
                for kp in range(KP):
                    p = ps_mm.tile([128, 2 * CH], F32, name=f"pb{kp}",
                                   tag="pmm")
                    for k in range(MT):
                        for m2 in range(2):
                            m = 2 * kp + m2
                            nc.tensor.matmul(
                                p[:, m2 * CH:(m2 + 1) * CH],
                                w2r[k][:, m * 128:(m + 1) * 128],
                                r1sl[k], start=(k == 0), stop=(k == MT - 1))
                    sq = sqp.tile([128, 2 * CH], F8, name=f"R2sq{kp}",
                                  tag=f"R2sq{kp}")
                    nc.scalar.activation(sq[:], p[:], AF.Square, scale=a2)
                    r = rp.tile([128, 2 * CH], F16, name=f"R2{kp}",
                                tag=f"R2{kp}")
                    emit_relu(r[:], p[:], kp)
                    R2.append(r); sq2.append(sq)
                    ra, sqa = l1_pair(next_x, kp)
                    next_out[0].append(ra); next_out[1].append(sqa)

            if cfg.get("dep_cut", False):
                R2 = [cst] * KP
                sq2 = [cst8] * KP
            st2 = stats_block(sq2, "2")
            t1 = sm.tile([1, CH], F32, name="t1", tag="t1")
            nc.vector.tensor_mul(t1[:], pv1[:], pv1[:])
            u1 = sm.tile([1, CH], F32, name="u1", tag="u1")
            nc.vector.tensor_mul(u1[:], t1[:], st2[0:1, :])
            pv2 = sm.tile([1, CH], F32, name="pv2", tag="pv2")
            nc.scalar.activation(pv2[:], u1[:], AF.Abs_reciprocal_sqrt,
                                 bias=eps_t[:], scale=1.0 / HID)
            q2 = sm.tile([1, CH], F32, name="q2", tag="q2")
            nc.vector.scalar_tensor_tensor(
                out=q2[:], in0=pv1[:], scalar=qd, in1=pv2[:],
                op0=ALU.mult, op1=ALU.mult)

            p3 = ps_p3.tile([1, CH], F32, name="p3", tag="p3")
            for k in range(MT):
                nc.tensor.matmul(p3[:], w3r[:, k:k + 1],
                                 R2[k // 2][:, (k % 2) * CH:(k % 2 + 1) * CH],
                                 start=(k == 0), stop=(k == MT - 1))
            osb = sm.tile([1, CH], F32, name="osb", tag="osb")
            nc.vector.tensor_mul(osb[:], p3[:], q2[:])
            if b3f != 0.0:
                nc.vector.tensor_scalar(out=osb[:], in0=osb[:], scalar1=b3f,
                                        scalar2=None, op0=ALU.add)
            nc.sync.dma_start(y.ap()[0:1, cs], osb[:])

        def _cs3(c):
            return slice(c * CH, (c + 1) * CH)

        def l2_pair(kp, r1sl):
            """One L2 pair-psum: 16 fp16 matmuls + sq2 + relu2."""
            p = ps_mm.tile([128, 2 * CH], F32, name=f"pb{kp}", tag="pmm")
            for k in range(MT):
                for m2 in range(2):
                    m = 2 * kp + m2
                    nc.tensor.matmul(p[:, m2 * CH:(m2 + 1) * CH],
                                     w2r[k][:, m * 128:(m + 1) * 128],
                                     r1sl[k], start=(k == 0),
                                     stop=(k == MT - 1))
            sq = sqp.tile([128, 2 * CH], F8, name=f"R2sq{kp}",
                          tag=f"R2sq{kp}")
            if cfg.get("swap_eng", False):
                nc.vector.tensor_mul(sq[:], p[:], p[:])
                r = rp.tile([128, 2 * CH], F16, name=f"R2{kp}",
                            tag=f"R2{kp}")
                nc.scalar.activation(r[:], p[:], AF.Relu)
            else:
                nc.scalar.activation(sq[:], p[:], AF.Square, scale=a2)
                r = rp.tile([128, 2 * CH], F16, name=f"R2{kp}",
                            tag=f"R2{kp}")
                emit_relu(r[:], p[:], kp)
            if cfg.get("dma_launder", False):
                rd = rp.tile([128, 2 * CH], F16, name=f"R2d{kp}",
                             tag=f"R2d{kp}")
                nc.sync.dma_start(rd[:], r[:])
                return rd, sq
            return r, sq

        def sched2_body():
            """Latency-aware PE order: stats/L3 placed where inputs are
            already computed; next-chunk L1 pairs used as PE filler."""
            x_cur = chunk_partA_dma(_cs3(0))
            R1 = []
            sq1 = []
            for kp in range(KP):
                r, sq = l1_pair(x_cur, kp)
                R1.append(r); sq1.append(sq)
            dc_relu = cfg.get("dep_cut_relu", False)
            dc4 = cfg.get("dep_cut_relu4", False)
            dc_sq = cfg.get("dep_cut_sq", False)
            if dc_relu or dc_sq or dc4:
                cst4 = []
                for i in range(KP):
                    t = const.tile([128, 2 * CH], F16, name=f"cst2_{i}",
                                   tag=f"cst2_{i}")
                    nc.vector.memset(t[:], 0.01)
                    cst4.append(t)
                cst2 = cst4[0]
                cst28 = const.tile([128, 2 * CH], F8, tag="cst28")
                nc.vector.memset(cst28[:], 0.01)
            for c in range(NCH):
                x_nxt = chunk_partA_dma(_cs3(c + 1)) if c + 1 < NCH else None
                if dc_relu:
                    R1 = [cst2] * KP
                elif dc4:
                    R1 = list(cst4)
                if dc_sq:
                    sq1 = [cst28] * KP
                r1sl = [R1[k // 2][:, (k % 2) * CH:(k % 2 + 1) * CH]
                        for k in range(MT)]
                R2, sq2 = [], []
                for kp in (0, 1):
                    r, sq = l2_pair(kp, r1sl)
                    R2.append(r); sq2.append(sq)
                st1 = stats_block(sq1, "1")
                pv1 = sm.tile([1, CH], F32, name="pv1", tag="pv1")
                nc.scalar.activation(pv1[:], st1[0:1, :],
                                     AF.Abs_reciprocal_sqrt,
                                     bias=eps_t[:], scale=1.0 / HID)
                for kp in (2, 3):
                    r, sq = l2_pair(kp, r1sl)
                    R2.append(r); sq2.append(sq)
                nR1, nsq1 = [], []
                if x_nxt is not None:
                    for kp in (0, 1):
                        r, sq = l1_pair(x_nxt, kp)
                        nR1.append(r); nsq1.append(sq)
                if dc_relu:
                    R2 = [cst2] * KP
                elif dc4:
                    R2 = list(cst4)
                if dc_sq:
                    sq2 = [cst28] * KP
                st2 = stats_block(sq2, "2")
                t1 = sm.tile([1, CH], F32, name="t1", tag="t1")
                nc.vector.tensor_mul(t1[:], pv1[:], pv1[:])
                u1 = sm.tile([1, CH], F32, name="u1", tag="u1")
                nc.vector.tensor_mul(u1[:], t1[:], st2[0:1, :])
                pv2 = sm.tile([1, CH], F32, name="pv2", tag="pv2")
                nc.scalar.activation(pv2[:], u1[:], AF.Abs_reciprocal_sqrt,
                                     bias=eps_t[:], scale=1.0 / HID)
                q2 = sm.tile([1, CH], F32, name="q2", tag="q2")
                nc.vector.scalar_tensor_tensor(
                    out=q2[:], in0=pv1[:], scalar=qd, in1=pv2[:],
                    op0=ALU.mult, op1=ALU.mult)
                if unified:
                    p3t = ps_mm.tile([128, 2 * CH], F32, name="p3t",
                                     tag="pmm")
                    p3a = p3t[0:1, 0:CH]
                else:
                    p3a = ps_p3.tile([1, CH], F32, name="p3", tag="p3")[:]
                for k in range(MT):
                    nc.tensor.matmul(
                        p3a, w3r[:, k:k + 1],
                        R2[k // 2][:, (k % 2) * CH:(k % 2 + 1) * CH],
                        start=(k == 0), stop=(k == MT - 1))
                osb = sm.tile([1, CH], F32, name="osb", tag="osb")
                nc.vector.tensor_mul(osb[:], p3a, q2[:])
                if b3f != 0.0:
                    nc.vector.tensor_scalar(out=osb[:], in0=osb[:],
                                            scalar1=b3f, scalar2=None,
                                            op0=ALU.add)
                nc.sync.dma_start(y.ap()[0:1, _cs3(c)], osb[:])
                if x_nxt is not None:
                    for kp in (2, 3):
                        r, sq = l1_pair(x_nxt, kp)
                        nR1.append(r); nsq1.append(sq)
                R1, sq1 = nR1, nsq1

        def whole_body():
            if cfg.get("sched2", True):
                sched2_body()
            elif cfg.get("pipe", False):
                # fine-grained software pipeline across chunks
                cur = chunk_partA(_cs3(0))
                for c in range(NCH):
                    if c + 1 < NCH:
                        nxt_x = chunk_partA_dma(_cs3(c + 1))
                        nxt = ([], [])
                        chunk_partB(_cs3(c), *cur, next_x=nxt_x,
                                    next_out=nxt)
                        cur = nxt
                    else:
                        chunk_partB(_cs3(c), *cur)
            elif cfg.get("skew", True):
                depth = cfg.get("skew_depth", 1)
                pend = [chunk_partA(_cs3(c)) for c in range(min(depth, NCH))]
                for c in range(NCH):
                    if c + depth < NCH:
                        pend.append(chunk_partA(_cs3(c + depth)))
                    chunk_partB(_cs3(c), *pend.pop(0))
            else:
                for c in range(NCH):
                    chunk_partB(_cs3(c), *chunk_partA(_cs3(c)))

        reps = cfg.get("unroll_reps")
        if reps:
            for _ in range(reps):
                whole_body()
        elif loop_iters is None:
            whole_body()
        else:
            with tc.For_i(0, loop_iters, 1):
                whole_body()
    nc.compile()
    return nc


def make_fast3_maps(inp):
    """Host prep for fast3. Returns (in_maps, scal)."""
    W1c, bc1, W2c, bc2 = _fold_weights(inp)
    g1 = np.asarray(inp["g1"], np.float64)
    g2 = np.asarray(inp["g2"], np.float64)
    W3 = np.asarray(inp["W3"], np.float32)
    b3 = np.asarray(inp["b3"], np.float32)

    XT = _build_xt_raw(inp)
    Xc = XT[128:185]
    XT[128:185] = np.sign(Xc) * np.log(np.abs(Xc) + 1.0)
    XTF = np.zeros((256, XT.shape[1]), np.float32)
    XTF[0:185] = XT
    XTF[185] = 1.0

    W1g = np.zeros((256, HID))
    W1g[0:185] = W1c.astype(np.float64) * g1[None, :]
    W1g[185] = bc1.astype(np.float64) * g1
    W2g = (W2c.astype(np.float64) * g2[None, :]).astype(np.float16)

    g1u = float(g1[0])
    g2u = float(g2[0])
    scal = (1.0 / g1u, 1.0 / g2u, 1.0, float(b3[0]))
    shared = {
        "w1": W1g.astype(np.float16), "w2": W2g,
        "w3p": _pack128(W3[:, 0]).astype(np.float16),
    }
    xp = _pack_rows(XTF.astype(np.float16), 2).reshape(128, 2, B)
    in_maps = []
    for c in range(N_CORES):
        m = dict(shared)
        sl = slice(c * B_CORE, (c + 1) * B_CORE)
        m["xt"] = np.ascontiguousarray(xp[:, :, sl]).reshape(128, 2 * B_CORE)
        in_maps.append(m)
    return in_maps, scal


# ---------------------------------------------------------------- fast4 device
# CH=1024 chunks (each m-tile psum spans 2 banks, written by 2 matmul
# half-chains). Halves the number of chunks and therefore the number of
# cross-engine PE waits, which cost ~1.5us each on HW regardless of slack.
# Contraction chains start at the newest-written rhs tile so a single
# watermark wait covers all eight.
CH4 = 1024
NCH4 = B_CORE // CH4


def build_fast4(scal, loop_iters=None, cfg=None):
    a1, a2, qd, b3f = scal
    cfg = cfg or {}
    nc = bacc.Bacc("TRN2", target_bir_lowering=False, debug=False)
    xt = nc.dram_tensor("xt", [128, 2 * B_CORE], F16, kind="ExternalInput")
    w1 = nc.dram_tensor("w1", [256, HID], F16, kind="ExternalInput")
    w2 = nc.dram_tensor("w2", [HID, HID], F16, kind="ExternalInput")
    w3p = nc.dram_tensor("w3p", [128, MT], F16, kind="ExternalInput")
    y = nc.dram_tensor("y", [1, B_CORE], F32, kind="ExternalOutput")

    from contextlib import ExitStack
    with tile.TileContext(nc) as tc, ExitStack() as ctx, \
            nc.allow_low_precision(reason="fp16/fp8 rounding is intentional"):
        const = ctx.enter_context(tc.tile_pool(name="const", bufs=1))
        xin = ctx.enter_context(tc.tile_pool(name="xin", bufs=3))
        rp = ctx.enter_context(tc.tile_pool(name="rp", bufs=2))
        sqp = ctx.enter_context(tc.tile_pool(name="sqp", bufs=2))
        sm = ctx.enter_context(tc.tile_pool(name="sm", bufs=4))
        ps_mm = ctx.enter_context(
            tc.tile_pool(name="ps_mm", bufs=3, space="PSUM"))
        ps_st = ctx.enter_context(
            tc.tile_pool(name="ps_st", bufs=1, space="PSUM"))
        ps_p3 = ctx.enter_context(
            tc.tile_pool(name="ps_p3", bufs=1, space="PSUM"))

        w1a = const.tile([128, HID], F16, tag="w1a")
        nc.sync.dma_start(w1a[:], w1.ap()[0:128, :])
        w1b = const.tile([128, HID], F16, tag="w1b")
        nc.sync.dma_start(w1b[:], w1.ap()[128:256, :])
        w2r = []
        for k in range(MT):
            t = const.tile([128, HID], F16, name=f"w2r{k}", tag=f"w2r{k}")
            nc.sync.dma_start(t[:], w2.ap()[k * 128:(k + 1) * 128, :])
            w2r.append(t)
        w3r = const.tile([128, MT], F16, tag="w3r")
        nc.sync.dma_start(w3r[:], w3p.ap())
        ones_st = const.tile([128, 2, 32], F32, tag="ones_st")
        nc.vector.memset(ones_st[:], 1.0)
        ones8 = const.tile([128, 2, 32], F8, tag="ones8")
        nc.vector.tensor_copy(ones8[:], ones_st[:])
        eps_t = const.tile([1, 1], F32, tag="eps_t")
        nc.vector.memset(eps_t[:], EPS)
        xt3 = xt.ap().rearrange("p (j b) -> p j b", j=2)

        def stats4(sqs, h, tag):
            """[32, 512] DRM stats over 4 sq-pair tiles, half h.
            Chain starts at the newest pair (index 3)."""
            stw = ps_st.tile([32, 512], F32, name=f"stw{tag}{h}", tag="stw")
            order = [3, 0, 1, 2]
            for i, kp in enumerate(order):
                nc.tensor.matmul(stw[:], ones8[:],
                                 sqs[kp][:, :, h * 512:(h + 1) * 512],
                                 start=(i == 0), stop=(i == 3),
                                 perf_mode=DRM)
            return stw

        def pv_of(st, tag):
            pv = sm.tile([1, 512], F32, name=f"pv{tag}", tag=f"pv{tag}")
            nc.scalar.activation(pv[:], st[0:1, :], AF.Abs_reciprocal_sqrt,
                                 bias=eps_t[:], scale=1.0 / HID)
            return pv

        def l1_m(x_t, m, sqt):
            p = ps_mm.tile([128, CH4], F32, name=f"pa{m}", tag="pmm")
            for h in range(2):
                for k in range(2):
                    nc.tensor.matmul(
                        p[:, h * 512 + 0:h * 512 + 512],
                        (w1a, w1b)[k][:, m * 128:(m + 1) * 128],
                        x_t[:, k, h * 512:(h + 1) * 512],
                        start=(k == 0), stop=(k == 1))
            nc.scalar.activation(sqt[m // 2][:, m % 2, :], p[:], AF.Square,
                                 scale=a1)
            r = rp.tile([128, CH4], F16, name=f"R1_{m}", tag=f"R1_{m}")
            nc.vector.tensor_scalar(out=r[:], in0=p[:], scalar1=0.0,
                                    scalar2=None, op0=ALU.max)
            return r

        def partA(c):
            x_t = xin.tile([128, 2, CH4], F16, name="x_t", tag="x_t")
            nc.sync.dma_start(x_t[:], xt3[:, :, c * CH4:(c + 1) * CH4])
            sqt = [sqp.tile([128, 2, CH4], F8, name=f"sq1_{i}",
                            tag=f"sq1_{i}") for i in range(4)]
            R1 = [l1_m(x_t, m, sqt) for m in range(MT)]
            return R1, sqt

        # k-order for L2/L3 chains: newest rhs tile first
        KORD = [7, 0, 1, 2, 3, 4, 5, 6]

        def partB(c, R1, sq1):
            st1 = [stats4(sq1, h, "1") for h in range(2)]
            pv1 = [pv_of(st1[h], f"1{h}") for h in range(2)]

            sq2t = [sqp.tile([128, 2, CH4], F8, name=f"sq2_{i}",
                             tag=f"sq2_{i}") for i in range(4)]
            R2 = []
            for m in range(MT):
                p = ps_mm.tile([128, CH4], F32, name=f"pb{m}", tag="pmm")
                for h in range(2):
                    for i, k in enumerate(KORD):
                        nc.tensor.matmul(
                            p[:, h * 512:h * 512 + 512],
                            w2r[k][:, m * 128:(m + 1) * 128],
                            R1[k][:, h * 512:(h + 1) * 512],
                            start=(i == 0), stop=(i == MT - 1))
                nc.scalar.activation(sq2t[m // 2][:, m % 2, :], p[:],
                                     AF.Square, scale=a2)
                r = rp.tile([128, CH4], F16, name=f"R2_{m}", tag=f"R2_{m}")
                nc.vector.tensor_scalar(out=r[:], in0=p[:], scalar1=0.0,
                                        scalar2=None, op0=ALU.max)
                R2.append(r)

            osb = sm.tile([1, CH4], F32, name="osb", tag="osb")
            for h in range(2):
                st2 = stats4(sq2t, h, "2")
                t1 = sm.tile([1, 512], F32, name=f"t1{h}", tag="t1")
                nc.vector.tensor_mul(t1[:], pv1[h][:], pv1[h][:])
                u1 = sm.tile([1, 512], F32, name=f"u1{h}", tag="u1")
                nc.vector.tensor_mul(u1[:], t1[:], st2[0:1, :])
                pv2 = pv_of(u1, f"2{h}")
                q2 = sm.tile([1, 512], F32, name=f"q2{h}", tag="q2")
                nc.vector.scalar_tensor_tensor(
                    out=q2[:], in0=pv1[h][:], scalar=qd, in1=pv2[:],
                    op0=ALU.mult, op1=ALU.mult)
                p3 = ps_p3.tile([1, 512], F32, name=f"p3{h}", tag="p3")
                for i, k in enumerate(KORD):
                    nc.tensor.matmul(p3[:], w3r[:, k:k + 1],
                                     R2[k][:, h * 512:(h + 1) * 512],
                                     start=(i == 0), stop=(i == MT - 1))
                ob = osb[:, h * 512:(h + 1) * 512]
                nc.vector.tensor_mul(ob, p3[:], q2[:])
                if b3f != 0.0:
                    nc.vector.tensor_scalar(out=ob, in0=ob, scalar1=b3f,
                                            scalar2=None, op0=ALU.add)
            nc.sync.dma_start(y.ap()[0:1, c * CH4:(c + 1) * CH4], osb[:])

        def whole_body():
            pend = partA(0)
            for c in range(NCH4):
                if c + 1 < NCH4:
                    nxt = partA(c + 1)
                else:
                    nxt = None
                partB(c, *pend)
                pend = nxt

        if loop_iters is None:
            whole_body()
        else:
            with tc.For_i(0, loop_iters, 1):
                whole_body()
    nc.compile()
    return nc


# ---------------------------------------------------------------- fast device
FAST_CFG = dict(xin_bufs=3, r_bufs=2, sq_bufs=2, sm_bufs=3,
                ps_mm_bufs=3, ps_st_bufs=2, ps_p3_bufs=2,
                relu_split=0, stats_late=True)


def build_fast(has_b3, loop_iters=None, cfg=None):
    """Fast-path program. has_b3: include final bias add."""
    cfg = {**FAST_CFG, **(cfg or {})}
    nc = bacc.Bacc("TRN2", target_bir_lowering=False, debug=False)
    xt = nc.dram_tensor("xt", [256, B_CORE], F16, kind="ExternalInput")
    w1 = nc.dram_tensor("w1", [256, HID], F16, kind="ExternalInput")
    w2 = nc.dram_tensor("w2", [HID, HID], F16, kind="ExternalInput")
    w3p = nc.dram_tensor("w3p", [128, MT], F16, kind="ExternalInput")
    s1p = nc.dram_tensor("s1p", [128, MT], F32, kind="ExternalInput")
    s2p = nc.dram_tensor("s2p", [128, MT], F32, kind="ExternalInput")
    b3t = nc.dram_tensor("b3t", [1, 1], F32, kind="ExternalInput")
    y = nc.dram_tensor("y", [1, B_CORE], F32, kind="ExternalOutput")

    from contextlib import ExitStack
    with tile.TileContext(nc) as tc, ExitStack() as ctx, \
            nc.allow_low_precision(reason="fp16/fp8 rounding is intentional"):
        const = ctx.enter_context(tc.tile_pool(name="const", bufs=1))
        xin = ctx.enter_context(tc.tile_pool(name="xin", bufs=cfg["xin_bufs"]))
        rp = ctx.enter_context(tc.tile_pool(name="rp", bufs=cfg["r_bufs"]))
        sqp = ctx.enter_context(tc.tile_pool(name="sqp", bufs=cfg["sq_bufs"]))
        sm = ctx.enter_context(tc.tile_pool(name="sm", bufs=cfg["sm_bufs"]))
        ps_mm = ctx.enter_context(
            tc.tile_pool(name="ps_mm", bufs=cfg["ps_mm_bufs"], space="PSUM"))
        ps_st = ctx.enter_context(
            tc.tile_pool(name="ps_st", bufs=cfg["ps_st_bufs"], space="PSUM"))
        ps_p3 = ctx.enter_context(
            tc.tile_pool(name="ps_p3", bufs=cfg["ps_p3_bufs"], space="PSUM"))

        # ---- one-time constants
        w1a = const.tile([128, HID], F16, tag="w1a")
        nc.sync.dma_start(w1a[:], w1.ap()[0:128, :])
        w1b = const.tile([128, HID], F16, tag="w1b")
        nc.sync.dma_start(w1b[:], w1.ap()[128:256, :])
        w2r = []
        for k in range(MT):
            t = const.tile([128, HID], F16, name=f"w2r{k}", tag=f"w2r{k}")
            nc.sync.dma_start(t[:], w2.ap()[k * 128:(k + 1) * 128, :])
            w2r.append(t)
        w3r = const.tile([128, MT], F16, tag="w3r")
        nc.sync.dma_start(w3r[:], w3p.ap())
        s1 = const.tile([128, MT], F32, tag="s1")
        nc.sync.dma_start(s1[:], s1p.ap())
        s2 = const.tile([128, MT], F32, tag="s2")
        nc.sync.dma_start(s2[:], s2p.ap())
        b3s = const.tile([1, 1], F32, tag="b3s")
        nc.sync.dma_start(b3s[:], b3t.ap())
        ones_st = const.tile([128, 2, 32], F32, tag="ones_st")
        nc.vector.memset(ones_st[:], 1.0)
        ones8 = const.tile([128, 2, 32], F8, tag="ones8")
        nc.vector.tensor_copy(ones8[:], ones_st[:])
        eps_t = const.tile([1, 1], F32, tag="eps_t")
        nc.vector.memset(eps_t[:], EPS)

        r_split = cfg.get("r_split", False)

        def layer_block(G_pool, w_tiles, rhs_list, sq_s, out_tag):
            """Emit MT m-tiles: matmuls + Square->fp8 pairs + relu->f16.
            Returns (R slices list of [128, CH] f16, sq pair tiles list)."""
            if r_split:
                Rs = [rp.tile([128, CH], F16, name=f"{out_tag}_{m}",
                              tag=f"{out_tag}_{m}") for m in range(MT)]
                rsl = [t[:] for t in Rs]
            else:
                R = rp.tile([128, MT * CH], F16, name=out_tag, tag=out_tag)
                rsl = [R[:, m * CH:(m + 1) * CH] for m in range(MT)]
            sqs = []
            for pr in range(MT // 2):
                sq = sqp.tile([128, 2, CH], F8, name=f"{out_tag}sq{pr}",
                              tag=f"{out_tag}sq{pr}")
                sqs.append(sq)
            il = cfg.get("mm_interleave", 1)
            nk = len(w_tiles)
            for m0 in range(0, MT, il):
                ms = list(range(m0, min(m0 + il, MT)))
                ptiles = [G_pool.tile([128, CH], F32, name=f"pmm{m}",
                                      tag="pmm") for m in ms]
                for k in range(nk):
                    for p, m in zip(ptiles, ms):
                        nc.tensor.matmul(
                            p[:], w_tiles[k][:, m * 128:(m + 1) * 128],
                            rhs_list[k], start=(k == 0), stop=(k == nk - 1))
                for p, m in zip(ptiles, ms):
                    nc.scalar.activation(sqs[m // 2][:, m % 2, :], p[:],
                                         AF.Square, scale=sq_s[:, m:m + 1])
                    nc.vector.tensor_scalar(out=rsl[m], in0=p[:], scalar1=0.0,
                                            scalar2=None, op0=ALU.max)
            return rsl, sqs

        def stats_block(sqs, tag):
            from contextlib import nullcontext
            off = cfg.get("stats_prio_off", 0)
            stw = ps_st.tile([32, CH], F32, name=f"stw{tag}", tag="stw")
            with (tc.high_priority(offset=off) if off else nullcontext()):
                for i, sq in enumerate(sqs):
                    nc.tensor.matmul(stw[:], ones8[:], sq[:], start=(i == 0),
                                     stop=(i == len(sqs) - 1), perf_mode=DRM)
            return stw

        def chunk_partA(cs):
            """DMA + layer 1 + LN1 elementwise (PE work available early)."""
            xa = xin.tile([128, CH], F16, name="xa", tag="xa")
            nc.sync.dma_start(xa[:], xt.ap()[0:128, cs])
            xb = xin.tile([128, CH], F16, name="xb", tag="xb")
            nc.sync.dma_start(xb[:], xt.ap()[128:256, cs])
            R1, sq1 = layer_block(ps_mm, [w1a, w1b], [xa[:], xb[:]], s1, "R1")
            return R1, sq1

        def chunk_partB(cs, R1, sq1):
            """stats1, layer 2, LN2, layer 3, output."""
            st1 = stats_block(sq1, "1")
            pv1 = sm.tile([1, CH], F32, name="pv1", tag="pv1")
            nc.scalar.activation(pv1[:], st1[0:1, :], AF.Abs_reciprocal_sqrt,
                                 bias=eps_t[:], scale=1.0 / HID)

            R2, sq2 = layer_block(ps_mm, w2r, list(R1), s2, "R2")
            st2 = stats_block(sq2, "2")
            t1 = sm.tile([1, CH], F32, name="t1", tag="t1")
            nc.vector.tensor_mul(t1[:], pv1[:], pv1[:])
            u1 = sm.tile([1, CH], F32, name="u1", tag="u1")
            nc.vector.tensor_mul(u1[:], t1[:], st2[0:1, :])
            pv2 = sm.tile([1, CH], F32, name="pv2", tag="pv2")
            nc.scalar.activation(pv2[:], u1[:], AF.Abs_reciprocal_sqrt,
                                 bias=eps_t[:], scale=1.0 / HID)
            q2 = sm.tile([1, CH], F32, name="q2", tag="q2")
            nc.vector.tensor_mul(q2[:], pv1[:], pv2[:])

            p3 = ps_p3.tile([1, CH], F32, name="p3", tag="p3")
            for k in range(MT):
                nc.tensor.matmul(p3[:], w3r[:, k:k + 1], R2[k],
                                 start=(k == 0), stop=(k == MT - 1))
            osb = sm.tile([1, CH], F32, name="osb", tag="osb")
            nc.vector.tensor_mul(osb[:], p3[:], q2[:])
            if has_b3:
                b3b = bass.AP(tensor=b3s[:].tensor, offset=b3s[:].offset,
                              ap=[b3s[:].ap[0], [0, CH]])
                nc.vector.tensor_tensor(out=osb[:], in0=osb[:], in1=b3b,
                                        op=ALU.add)
            nc.sync.dma_start(y.ap()[0:1, cs], osb[:])

        def _cs(c):
            return slice(c * CH, (c + 1) * CH)

        def whole_body():
            cl = cfg.get("chunk_loop")
            if cl is not None:
                unroll = cfg.get("chunk_unroll", 1)
                hint = ((mybir.EngineType.PE,)
                        if cfg.get("hint_pe", False) else ())
                stag = cfg.get("staggered_reset", False)
                with tc.For_i(0, NCH // unroll, 1, hint_engines=hint,
                              staggered_reset=stag) as iv:
                    for u in range(unroll):
                        cs = bass.ds(iv * (CH * unroll) + u * CH, CH)
                        chunk_partB(cs, *chunk_partA(cs))
            elif cfg.get("skew", True):
                depth = cfg.get("skew_depth", 1)
                pend = [chunk_partA(_cs(c)) for c in range(min(depth, NCH))]
                for c in range(NCH):
                    if c + depth < NCH:
                        pend.append(chunk_partA(_cs(c + depth)))
                    chunk_partB(_cs(c), *pend.pop(0))
            else:
                for c in range(NCH):
                    chunk_partB(_cs(c), *chunk_partA(_cs(c)))

        if loop_iters is None:
            whole_body()
        else:
            with tc.For_i(0, loop_iters, 1):
                whole_body()
    nc.compile()
    return nc


def make_fast_maps(inp):
    """Host prep for the fast path. Returns (in_maps, has_b3)."""
    W1c, bc1, W2c, bc2 = _fold_weights(inp)
    g1 = np.asarray(inp["g1"], np.float64)
    g2 = np.asarray(inp["g2"], np.float64)
    W3 = np.asarray(inp["W3"], np.float32)
    b3 = np.asarray(inp["b3"], np.float32)

    XT = _build_xt_raw(inp)
    Xc = XT[128:185]
    XT[128:185] = np.sign(Xc) * np.log(np.abs(Xc) + 1.0)
    XTF = np.zeros((256, XT.shape[1]), np.float16)
    XTF[0:185] = XT.astype(np.float16)
    XTF[185] = 1.0

    W1g = (W1c.astype(np.float64) * g1[None, :])
    bc1g = bc1.astype(np.float64) * g1
    W1full = np.zeros((256, HID), np.float16)
    W1full[0:185] = W1g.astype(np.float16)
    W1full[185] = bc1g.astype(np.float16)
    W2g = (W2c.astype(np.float64) * g2[None, :]).astype(np.float16)

    shared = {
        "w1": W1full, "w2": W2g,
        "w3p": _pack128(W3[:, 0]).astype(np.float16),
        "s1p": _pack128(1.0 / g1), "s2p": _pack128(1.0 / g2),
        "b3t": b3.reshape(1, 1).astype(np.float32),
    }
    in_maps = []
    for c in range(N_CORES):
        m = dict(shared)
        m["xt"] = np.ascontiguousarray(XTF[:, c * B_CORE:(c + 1) * B_CORE])
        in_maps.append(m)
    return in_maps, bool(np.any(b3 != 0.0))


def fast_path_ok(inp):
    be1 = np.asarray(inp["be1"]); be2 = np.asarray(inp["be2"])
    g1 = np.asarray(inp["g1"]); g2 = np.asarray(inp["g2"])
    _, _, _, bc2 = _fold_weights(inp)
    return (np.all(be1 == 0.0) and np.all(be2 == 0.0)
            and np.all(np.abs(bc2) < 1e-12)
            and np.all(np.abs(g1) > 1e-6) and np.all(np.abs(g2) > 1e-6))


# ---------------------------------------------------------------- legacy path
DEFAULT_CFG = dict(h_bufs=1, sq_bufs=1, r1_bufs=1, r2_bufs=1,
                   ps_mm_bufs=3, xin_bufs=3, xr_bufs=2, per_m=False,
                   l2_fp16=False, h_fp16=False)


def build_program(simple_affine, loop_iters=None, cfg=None):
    """Legacy general-path program (arbitrary affine/bias)."""
    cfg = {**DEFAULT_CFG, **(cfg or {})}
    nc = bacc.Bacc("TRN2", target_bir_lowering=False, debug=False)
    xt = nc.dram_tensor("xt", [KA + KC, B_CORE], F32, kind="ExternalInput")
    w1 = nc.dram_tensor("w1", [KA + KC, HID], F32, kind="ExternalInput")
    w2 = nc.dram_tensor("w2", [HID, HID], F32, kind="ExternalInput")
    w3p = nc.dram_tensor("w3p", [128, MT], F32, kind="ExternalInput")
    bc1p = nc.dram_tensor("bc1p", [128, MT], F32, kind="ExternalInput")
    bc2p = nc.dram_tensor("bc2p", [128, MT], F32, kind="ExternalInput")
    g1p = nc.dram_tensor("g1p", [128, MT], F32, kind="ExternalInput")
    be1p = nc.dram_tensor("be1p", [128, MT], F32, kind="ExternalInput")
    g2p = nc.dram_tensor("g2p", [128, MT], F32, kind="ExternalInput")
    be2p = nc.dram_tensor("be2p", [128, MT], F32, kind="ExternalInput")
    b3t = nc.dram_tensor("b3t", [1, 1], F32, kind="ExternalInput")
    y = nc.dram_tensor("y", [1, B_CORE], F32, kind="ExternalOutput")

    from contextlib import ExitStack
    with tile.TileContext(nc) as tc, ExitStack() as ctx, \
            nc.allow_low_precision(reason="f32r rounding is intentional"):
        const = ctx.enter_context(tc.tile_pool(name="const", bufs=1))
        wstage = ctx.enter_context(tc.tile_pool(name="wstage", bufs=2))
        xin = ctx.enter_context(tc.tile_pool(name="xin", bufs=cfg["xin_bufs"]))
        xr = ctx.enter_context(tc.tile_pool(name="xr", bufs=cfg["xr_bufs"]))
        bigH = ctx.enter_context(tc.tile_pool(name="bigH", bufs=cfg["h_bufs"]))
        bigS = ctx.enter_context(tc.tile_pool(name="bigS", bufs=cfg["sq_bufs"]))
        bigR1 = ctx.enter_context(tc.tile_pool(name="bigR1", bufs=cfg["r1_bufs"]))
        bigR2 = ctx.enter_context(tc.tile_pool(name="bigR2", bufs=cfg["r2_bufs"]))
        small = ctx.enter_context(tc.tile_pool(name="small", bufs=cfg.get("small_bufs", 2)))
        ps_mm = ctx.enter_context(tc.tile_pool(name="ps_mm", bufs=cfg["ps_mm_bufs"], space="PSUM"))
        ps_st = ctx.enter_context(tc.tile_pool(name="ps_st", bufs=cfg.get("ps_st_bufs", 2), space="PSUM"))
        ps_vec = ctx.enter_context(tc.tile_pool(name="ps_vec", bufs=cfg.get("ps_vec_bufs", 2), space="PSUM"))

        w1a_r = const.tile([128, HID], F32R, tag="w1a")
        st = wstage.tile([128, HID], F32, tag="stage")
        nc.sync.dma_start(st[:], w1.ap()[0:128, :])
        nc.vector.tensor_copy(w1a_r[:], st[:])
        w1c_r = const.tile([KC, HID], F32R, tag="w1c")
        stc = wstage.tile([KC, HID], F32, tag="stagec")
        nc.sync.dma_start(stc[:], w1.ap()[128:185, :])
        nc.vector.tensor_copy(w1c_r[:], stc[:])
        L2DT = mybir.dt.float16 if cfg["l2_fp16"] else F32R
        w2r = []
        for k in range(MT):
            stk = wstage.tile([128, HID], F32, tag="stage")
            nc.sync.dma_start(stk[:], w2.ap()[k * 128:(k + 1) * 128, :])
            t = const.tile([128, HID], L2DT, tag=f"w2r{k}")
            nc.vector.tensor_copy(t[:], stk[:])
            w2r.append(t)
        w3p_r = const.tile([128, MT], L2DT, tag="w3p")
        st3 = wstage.tile([128, MT], F32, tag="stages")
        nc.sync.dma_start(st3[:], w3p.ap())
        nc.vector.tensor_copy(w3p_r[:], st3[:])

        def load_small(name, dram):
            t = const.tile([128, MT], F32, tag=name)
            nc.sync.dma_start(t[:], dram.ap())
            return t
        bc1s = load_small("bc1s", bc1p); bc2s = load_small("bc2s", bc2p)
        g1s = load_small("g1s", g1p); be1s = load_small("be1s", be1p)
        g2s = load_small("g2s", g2p); be2s = load_small("be2s", be2p)
        b3s = const.tile([1, 1], F32, tag="b3s")
        nc.sync.dma_start(b3s[:], b3t.ap())
        ones_st = const.tile([128, 1], F32, tag="ones_st")
        nc.vector.memset(ones_st[:], 1.0)
        ones_col = const.tile([128, 1], F32R, tag="ones_col")
        nc.vector.tensor_copy(ones_col[:], ones_st[:])
        ones_rst = const.tile([1, 128], F32, tag="ones_rst")
        nc.vector.memset(ones_rst[:], 1.0)
        ones_row = const.tile([1, 128], F32R, tag="ones_row")
        nc.vector.tensor_copy(ones_row[:], ones_rst[:])
        eps_t = const.tile([1, 1], F32, tag="eps_t")
        nc.vector.memset(eps_t[:], EPS)

        def layer_norm_relu(Hb, g_s, be_s, out_pool, out_tag):
            sqb = bigS.tile([128, MT * CH], F32R, tag="sq")
            if cfg["per_m"]:
                for m in range(MT):
                    sl = slice(m * CH, (m + 1) * CH)
                    nc.vector.tensor_mul(sqb[:, sl], Hb[:, sl], Hb[:, sl])
            else:
                nc.vector.tensor_mul(sqb[:], Hb[:], Hb[:])
            pst = ps_st.tile([1, CH], F32, tag="pst")
            for m in range(MT):
                nc.tensor.matmul(pst[:], ones_col[:],
                                 sqb[:, m * CH:(m + 1) * CH],
                                 start=(m == 0), stop=(m == MT - 1))
            sd = small.tile([1, CH], F32, tag="sd")
            nc.scalar.activation(sd[:], pst[:], AF.Sqrt,
                                 bias=eps_t[:], scale=1.0 / HID)
            rs = small.tile([1, CH], F32R, tag="rs")
            nc.vector.reciprocal(rs[:], sd[:])
            pv = ps_vec.tile([128, CH], F32, tag="pv")
            nc.tensor.matmul(pv[:], ones_row[:], rs[:], start=True, stop=True)
            Rb = out_pool.tile([128, MT * CH], L2DT, tag=out_tag)
            if cfg["per_m"]:
                for m in range(MT):
                    sl = slice(m * CH, (m + 1) * CH)
                    nc.vector.tensor_mul(Hb[:, sl], Hb[:, sl], pv[:])
                    if simple_affine:
                        nc.scalar.activation(Rb[:, sl], Hb[:, sl], AF.Relu)
                    else:
                        nc.scalar.activation(Rb[:, sl], Hb[:, sl], AF.Relu,
                                             bias=be_s[:, m:m + 1],
                                             scale=g_s[:, m:m + 1])
            else:
                h3 = Hb[:].rearrange("p (m n) -> p m n", m=MT)
                pvb = bass.AP(tensor=pv[:].tensor, offset=pv[:].offset,
                              ap=[pv[:].ap[0], [0, MT], pv[:].ap[1]])
                nc.vector.tensor_mul(h3, h3, pvb)
                if simple_affine:
                    nc.scalar.activation(Rb[:], Hb[:], AF.Relu)
                else:
                    for m in range(MT):
                        sl = slice(m * CH, (m + 1) * CH)
                        nc.scalar.activation(Rb[:, sl], Hb[:, sl], AF.Relu,
                                             bias=be_s[:, m:m + 1],
                                             scale=g_s[:, m:m + 1])
            return Rb

        HDT = mybir.dt.float16 if cfg["h_fp16"] else F32

        def chunk_body(c):
            x1 = xin.tile([128, CH], F32, tag="x1")
            nc.sync.dma_start(x1[:], xt.ap()[0:128, c * CH:(c + 1) * CH])
            x2 = xin.tile([KC, CH], F32, tag="x2")
            nc.sync.dma_start(x2[:], xt.ap()[128:185, c * CH:(c + 1) * CH])
            x1r = xr.tile([128, CH], F32R, tag="x1r")
            nc.vector.tensor_copy(x1r[:], x1[:])
            xab = xr.tile([KC, CH], F32, tag="xab")
            nc.vector.tensor_scalar(
                out=xab[:].bitcast(mybir.dt.int32),
                in0=x2[:].bitcast(mybir.dt.int32),
                scalar1=0x7FFFFFFF, scalar2=None, op0=ALU.bitwise_and)
            xln = xr.tile([KC, CH], F32, tag="xln")
            nc.scalar.activation(xln[:], xab[:], AF.Ln, bias=1.0)
            xsg = xr.tile([KC, CH], F32, tag="xsg")
            nc.scalar.activation(xsg[:], x2[:], AF.Sign)
            x2r = xr.tile([KC, CH], F32R, tag="x2r")
            nc.vector.tensor_mul(x2r[:], xsg[:], xln[:])

            H1 = bigH.tile([128, MT * CH], HDT, tag="H")
            for m in range(MT):
                p1 = ps_mm.tile([128, CH], F32, tag="pmm")
                nc.tensor.matmul(p1[:], w1a_r[:, m * 128:(m + 1) * 128],
                                 x1r[:], start=True, stop=False)
                nc.tensor.matmul(p1[:], w1c_r[:, m * 128:(m + 1) * 128],
                                 x2r[:], start=False, stop=True)
                nc.scalar.activation(H1[:, m * CH:(m + 1) * CH], p1[:],
                                     AF.Identity, bias=bc1s[:, m:m + 1])
            R1 = layer_norm_relu(H1, g1s, be1s, bigR1, "R1")

            H2 = bigH.tile([128, MT * CH], HDT, tag="H")
            for m in range(MT):
                p2 = ps_mm.tile([128, CH], F32, tag="pmm")
                for k in range(MT):
                    nc.tensor.matmul(p2[:], w2r[k][:, m * 128:(m + 1) * 128],
                                     R1[:, k * CH:(k + 1) * CH],
                                     start=(k == 0), stop=(k == MT - 1))
                nc.scalar.activation(H2[:, m * CH:(m + 1) * CH], p2[:],
                                     AF.Identity, bias=bc2s[:, m:m + 1])
            R2 = layer_norm_relu(H2, g2s, be2s, bigR2, "R2")

            p3 = ps_st.tile([1, CH], F32, tag="pst")
            for k in range(MT):
                nc.tensor.matmul(p3[:], w3p_r[:, k:k + 1],
                                 R2[:, k * CH:(k + 1) * CH],
                                 start=(k == 0), stop=(k == MT - 1))
            osb = small.tile([1, CH], F32, tag="osb")
            nc.scalar.activation(osb[:], p3[:], AF.Identity, bias=b3s[:])
            nc.sync.dma_start(y.ap()[0:1, c * CH:(c + 1) * CH], osb[:])

        if loop_iters is None:
            for c in range(NCH):
                chunk_body(c)
        else:
            with tc.For_i(0, loop_iters, 1):
                for c in range(NCH):
                    chunk_body(c)
    nc.compile()
    return nc


def _build_xt_legacy(inp):
    XT = np.zeros((KA + KC, inp["op_vec"].shape[0]), np.float32)
    XT[0:185] = _build_xt_raw(inp)
    return XT


def make_legacy_maps(inp):
    W1c, bc1, W2c, bc2 = _fold_weights(inp)
    XT = _build_xt_legacy(inp)
    g1 = np.asarray(inp["g1"], np.float32); be1 = np.asarray(inp["be1"], np.float32)
    g2 = np.asarray(inp["g2"], np.float32); be2 = np.asarray(inp["be2"], np.float32)
    simple_affine = bool(
        np.all(g1 == 1.0) and np.all(g2 == 1.0)
        and np.all(be1 == 0.0) and np.all(be2 == 0.0))
    W3 = np.asarray(inp["W3"], np.float32)
    b3 = np.asarray(inp["b3"], np.float32)
    shared = {
        "w1": W1c, "w2": W2c,
        "w3p": _pack128(W3[:, 0]),
        "bc1p": _pack128(bc1), "bc2p": _pack128(bc2),
        "g1p": _pack128(g1), "be1p": _pack128(be1),
        "g2p": _pack128(g2), "be2p": _pack128(be2),
        "b3t": b3.reshape(1, 1),
    }
    in_maps = []
    for c in range(N_CORES):
        m = dict(shared)
        m["xt"] = np.ascontiguousarray(XT[:, c * B_CORE:(c + 1) * B_CORE])
        in_maps.append(m)
    return in_maps, simple_affine


# ---------------------------------------------------------------- entry point
_CACHE = {}

BEST_CFG = dict(ps_mm_bufs=5, ps_st_bufs=2, ps_p3_bufs=1)
LEGACY_BEST_CFG = dict(per_m=True, h_bufs=2, ps_mm_bufs=4)


def make_in_maps(inputs):
    """Returns (in_maps, mode) with mode = ("fast2", scal),
    ("fast", has_b3) or ("legacy", simple_affine)."""
    inp = {k: np.asarray(v) for k, v in inputs.items()}
    if fast2_ok(inp):
        in_maps, scal = make_fast3_maps(inp)
        return in_maps, ("fast3", scal)
    if fast_path_ok(inp):
        in_maps, has_b3 = make_fast_maps(inp)
        return in_maps, ("fast", has_b3)
    in_maps, simple_affine = make_legacy_maps(inp)
    return in_maps, ("legacy", simple_affine)


def build_for_mode(mode, loop_iters=None, cfg=None):
    kind, flag = mode
    if kind == "fast4":
        return build_fast4(flag, loop_iters=loop_iters, cfg=cfg)
    if kind == "fast3":
        return build_fast3(flag, loop_iters=loop_iters, cfg=cfg)
    if kind == "fast2":
        return build_fast2(flag, loop_iters=loop_iters, cfg=cfg)
    if kind == "fast":
        return build_fast(flag, loop_iters=loop_iters,
                          cfg=cfg if cfg is not None else BEST_CFG)
    return build_program(flag, loop_iters=loop_iters,
                         cfg=cfg if cfg is not None else LEGACY_BEST_CFG)


def _get_program(mode):
    key = ("prog", mode)
    if key not in _CACHE:
        _CACHE[key] = build_for_mode(mode)
    return _CACHE[key]


def kernel(**inputs) -> np.ndarray:
    in_maps, mode = make_in_maps(inputs)
    nc = _get_program(mode)
    res = run_bass_kernel_spmd(nc, in_maps, core_ids=list(range(N_CORES)))
    y = np.concatenate([r["y"][0] for r in res.results])
    return y.reshape(B, 1).astype(np.float32)


if __name__ == "__main__":
    import jax
    import reference
    cpu = jax.devices("cpu")[0]
    with jax.default_device(cpu):
        inp = reference.setup_inputs()
        ref = np.asarray(reference.reference(**inp))
    out = kernel(**{k: np.asarray(v) for k, v in inp.items()})
    err = np.abs(out - ref)
    scale = np.abs(ref).max()
    print("max_abs", err.max(), "rel(vs scale)", err.max() / scale,
          "mean_rel", (err / (np.abs(ref) + 1e-6)).mean())



# revision 39
# speedup vs baseline: 1.0011x; 1.0011x over previous
"""Trainium2 Bass kernel for nn_Autotuner_FFN (dense MLP, 8-core data parallel).

Fast-path structure (be1=be2=0, bc2=0 — true for this model's inputs):
  * Host precomputes the feature matrix XT [256, B] in fp16: one-hot
    encodings, the 57 sign(x)*ln(|x|+1) transformed features, a ones row
    carrying the folded first-layer bias, zero padding to 2 full K=128
    tiles. LayerNorm affine g is folded into W1/W2 columns (stats use a
    per-partition 1/g prescale); mean-centering is folded into weights.
  * Per 512-sample chunk the device does only:
      L1: 16 fp16 matmuls -> G1 (PSUM)
      LN1: Act Square(G1)->fp8 pairs, DVE relu(G1)->f16 R1 (pv1 DEFERRED)
      stats1: 4 fp8 DoubleRow ones-reduce matmuls; pv1 = AbsRsqrt LUT
      L2: 64 fp16 matmuls over unnormalized R1 -> G2
      LN2: same; variance rescaled by pv1^2 in [1,512] smalls
      L3: 8 fp16 matmuls -> g3; y = pv1*pv2*g3 (+b3)
    LayerNorm scale-invariance makes the deferral exact: relu(c*x) =
    c*relu(x) for c>0, so per-column scales commute out to the end.
  * No PSUM->SBUF copies, no rsqrt broadcast matmuls, no bias adds, no
    device transcendentals except the one AbsRsqrt LUT per LN.
  * All matmul K-tiles are padded to 128 (K<128 matmuls run ~2x slower).
Legacy general path (arbitrary affine/bias) retained below.
"""
import numpy as np

import concourse.bass as bass
import concourse.tile as tile
from concourse import bacc, mybir
from concourse.bass_utils import run_bass_kernel_spmd

AF = mybir.ActivationFunctionType
ALU = mybir.AluOpType
F32 = mybir.dt.float32
F16 = mybir.dt.float16
F8 = mybir.dt.float8e4
DRM = mybir.MatmulPerfMode.DoubleRow
F32R = mybir.dt.float16  # legacy alias

B = 65536
N_CORES = 8
B_CORE = B // N_CORES          # 8192
CH = 512                       # batch chunk (one PSUM bank wide)
NCH = B_CORE // CH             # 16
HID = 1024
MT = HID // 128                # 8 hidden m-tiles
KA, KC = 128, 57               # legacy feature K tiles
EPS = 1e-5
LN2 = float(np.log(2.0))


# ---------------------------------------------------------------- host folds
def _fold_weights(inp):
    f8 = lambda x: np.asarray(x, np.float64)
    W1 = f8(inp["W1"]); b1 = f8(inp["b1"])
    emb_kc = f8(inp["emb_kc"]); emb_nl = f8(inp["emb_nl"])
    op_W = f8(inp["op_W"]); op_b = f8(inp["op_b"])
    emb_c = f8(inp["emb_contig"]); emb_s = f8(inp["emb_scalar"])
    emb_i = f8(inp["emb_indirect"])
    H = W1.shape[1]
    rows_A = []
    bias = b1.copy()
    rows_A.append(emb_kc @ W1[0:16])
    rows_A.append(emb_nl @ W1[16:32])
    W1_op = W1[32:944].reshape(57, 16, H)
    rows_A.append(np.einsum("ij,ijh->ih", op_W, W1_op))
    bias += np.einsum("ij,ijh->h", op_b, W1_op)
    rd_f2, rd_bool, rd_ss = [], [], []
    wd_f2, wd_bool, wd_ss = [], [], []
    for base, f2l, booll, ssl in ((947, rd_f2, rd_bool, rd_ss),
                                  (1027, wd_f2, wd_bool, wd_ss)):
        for d in range(4):
            Wd = W1[base + 20 * d: base + 20 * d + 20]
            f2l.append(Wd[0:2])
            ssl.append(Wd[2:8] / LN2)
            rows_b = []
            for e, sl in ((emb_c, slice(8, 12)), (emb_s, slice(12, 16)),
                          (emb_i, slice(16, 20))):
                rows_b.append((e[1] - e[0]) @ Wd[sl])
                bias += e[0] @ Wd[sl]
            booll.append(np.stack(rows_b))
    rows_A += [np.concatenate(rd_f2), np.concatenate(rd_bool),
               np.concatenate(wd_f2), np.concatenate(wd_bool),
               W1[1110:1112]]
    A = np.concatenate(rows_A)
    C = np.concatenate([W1[944:947] / LN2, W1[1107:1110] / LN2,
                        W1[1112:1115] / LN2,
                        np.concatenate(rd_ss), np.concatenate(wd_ss)])
    W1_eff = np.concatenate([A, np.zeros((3, H)), C])       # [185, H]
    W1c = W1_eff - W1_eff.mean(axis=1, keepdims=True)
    bc1 = bias - bias.mean()
    W2 = f8(inp["W2"]); b2 = f8(inp["b2"])
    W2c = W2 - W2.mean(axis=1, keepdims=True)
    bc2 = b2 - b2.mean()
    return (W1c.astype(np.float32), bc1.astype(np.float32),
            W2c.astype(np.float32), bc2.astype(np.float32))


def _build_xt_raw(inp):
    """[185, B] float32 feature matrix, 57 transform rows still raw."""
    Bn = inp["op_vec"].shape[0]
    kc = np.asarray(inp["kernel_category_idx"]).astype(np.int64)
    nl = np.asarray(inp["num_of_loops_idx"]).astype(np.int64)
    f = lambda k: np.asarray(inp[k], np.float32)
    XT = np.zeros((185, Bn), np.float32)
    XT[0:10] = (np.arange(10)[:, None] == kc[None, :])
    XT[10:26] = (np.arange(16)[:, None] == nl[None, :])
    XT[26:83] = f("op_vec").T
    XT[83:91] = f("read_dep_float")[:, :, 0:2].reshape(Bn, 8).T
    XT[91:103] = np.asarray(inp["read_dep_bools"]).reshape(Bn, 12).T
    XT[103:111] = f("write_dep_float")[:, :, 0:2].reshape(Bn, 8).T
    XT[111:123] = np.asarray(inp["write_dep_bools"]).reshape(Bn, 12).T
    XT[123:125] = f("rest_vec")[:, 3:5].T
    XT[128:131] = f("size_hints").T
    XT[131:137] = f("rest_vec")[:, [0, 1, 2, 5, 6, 7]].T
    XT[137:161] = f("read_dep_float")[:, :, 2:8].reshape(Bn, 24).T
    XT[161:185] = f("write_dep_float")[:, :, 2:8].reshape(Bn, 24).T
    return XT


def _pack128(v):
    """[1024] -> [128, 8] with v[m*128+p] at [p, m]."""
    return np.ascontiguousarray(np.asarray(v, np.float32).reshape(8, 128).T)


# ---------------------------------------------------------------- fast2 device
# fp8 hi+lo split-matmul path. All matmuls (except L3) run as fp8e4m3
# DoubleRow 3-pass Karatsuba: A@B ~ Ah@Bh + Al@Bh + Ah@Bl, with operands
# pre-scaled by powers of two so every fp8 value sits in the normal range
# (the naive split fails because W values ~0.03 put the lo term in
# subnormals). Scale domains: X*SX, W1*SW1 -> G1 psum = D1*g1*h1;
# W2*SW2 -> G2 psum = D2*g2*h2'. relu-hi/lo extraction then needs no
# scaling at all: hi = fp8(max(G,0)), lo = fp8(max(G,0)-hi).
SX, SW1, SW2 = 1.0, 16.0, 32.0
D1 = SX * SW1
D2 = D1 * SW2

FAST2_CFG = dict(xin_bufs=3, rh_bufs=2, rl_bufs=2, sq_bufs=2, r2_bufs=2,
                 sm_bufs=3, ps_mm_bufs=3, ps_st_bufs=1, ps_p3_bufs=1,
                 skew=True, skew_depth=1)


def build_fast2(scal, loop_iters=None, cfg=None):
    """scal = (a1, a2, qd, b3f): Square scales for LN1/LN2, final dequant
    pv1*pv2*qd, final bias (baked)."""
    a1, a2, qd, b3f = scal
    cfg = {**FAST2_CFG, **(cfg or {})}
    nc = bacc.Bacc("TRN2", target_bir_lowering=False, debug=False)
    xh = nc.dram_tensor("xh", [128, 2 * B_CORE], F8, kind="ExternalInput")
    xl = nc.dram_tensor("xl", [128, 2 * B_CORE], F8, kind="ExternalInput")
    w1h = nc.dram_tensor("w1h", [128, 2 * HID], F8, kind="ExternalInput")
    w1l = nc.dram_tensor("w1l", [128, 2 * HID], F8, kind="ExternalInput")
    w2h = nc.dram_tensor("w2h", [128, 8 * HID], F8, kind="ExternalInput")
    w2l = nc.dram_tensor("w2l", [128, 8 * HID], F8, kind="ExternalInput")
    w3p = nc.dram_tensor("w3p", [128, MT], F16, kind="ExternalInput")
    y = nc.dram_tensor("y", [1, B_CORE], F32, kind="ExternalOutput")
    KP = 4                                      # hidden 128-row pair tiles

    from contextlib import ExitStack
    with tile.TileContext(nc) as tc, ExitStack() as ctx, \
            nc.allow_low_precision(reason="fp8 hi/lo split is intentional"):
        const = ctx.enter_context(tc.tile_pool(name="const", bufs=1))
        xin = ctx.enter_context(tc.tile_pool(name="xin", bufs=cfg["xin_bufs"]))
        rhp = ctx.enter_context(tc.tile_pool(name="rhp", bufs=cfg["rh_bufs"]))
        rlp = ctx.enter_context(tc.tile_pool(name="rlp", bufs=cfg["rl_bufs"]))
        sqp = ctx.enter_context(tc.tile_pool(name="sqp", bufs=cfg["sq_bufs"]))
        r2p = ctx.enter_context(tc.tile_pool(name="r2p", bufs=cfg["r2_bufs"]))
        sm = ctx.enter_context(tc.tile_pool(name="sm", bufs=cfg["sm_bufs"]))
        ps_mm = ctx.enter_context(
            tc.tile_pool(name="ps_mm", bufs=cfg["ps_mm_bufs"], space="PSUM"))
        ps_st = ctx.enter_context(
            tc.tile_pool(name="ps_st", bufs=cfg["ps_st_bufs"], space="PSUM"))
        ps_p3 = ctx.enter_context(
            tc.tile_pool(name="ps_p3", bufs=cfg["ps_p3_bufs"], space="PSUM"))

        def load_const(name, dram, shape, dt):
            t = const.tile(shape, dt, tag=name)
            nc.sync.dma_start(t[:], dram.ap())
            return t
        w1h_t = load_const("w1h_t", w1h, [128, 2 * HID], F8)
        w1l_t = load_const("w1l_t", w1l, [128, 2 * HID], F8)
        w2h_t = load_const("w2h_t", w2h, [128, 8 * HID], F8)
        w2l_t = load_const("w2l_t", w2l, [128, 8 * HID], F8)
        w3r = load_const("w3r", w3p, [128, MT], F16)
        ones_st = const.tile([128, 2, 32], F32, tag="ones_st")
        nc.vector.memset(ones_st[:], 1.0)
        ones8 = const.tile([128, 2, 32], F8, tag="ones8")
        nc.vector.tensor_copy(ones8[:], ones_st[:])
        eps_t = const.tile([1, 1], F32, tag="eps_t")
        nc.vector.memset(eps_t[:], EPS)

        w1h3 = w1h_t[:].rearrange("p (j n) -> p j n", j=2)
        w1l3 = w1l_t[:].rearrange("p (j n) -> p j n", j=2)
        w2h4 = w2h_t[:].rearrange("p (k j n) -> p k j n", k=KP, j=2)
        w2l4 = w2l_t[:].rearrange("p (k j n) -> p k j n", k=KP, j=2)
        xh3 = xh.ap().rearrange("p (j b) -> p j b", j=2)
        xl3 = xl.ap().rearrange("p (j b) -> p j b", j=2)

        def stats_block(sqs, tag):
            stw = ps_st.tile([32, CH], F32, name=f"stw{tag}", tag="stw")
            for i, sq in enumerate(sqs):
                nc.tensor.matmul(stw[:], ones8[:],
                                 sq[:].rearrange("p (j c) -> p j c", j=2),
                                 start=(i == 0), stop=(i == len(sqs) - 1),
                                 perf_mode=DRM)
            return stw

        def chunk_partA(cs):
            """DMA x hi/lo, L1 3-pass DRM matmuls, LN1 elementwise."""
            xh_t = xin.tile([128, 2, CH], F8, name="xh_t", tag="xh_t")
            nc.sync.dma_start(xh_t[:], xh3[:, :, cs])
            xl_t = xin.tile([128, 2, CH], F8, name="xl_t", tag="xl_t")
            nc.sync.dma_start(xl_t[:], xl3[:, :, cs])
            his, los, sqs = [], [], []
            for kp in range(KP):
                p = ps_mm.tile([128, 2 * CH], F32, name="pmm", tag="pmm")
                for m2 in range(2):
                    sl = slice((2 * kp + m2) * 128, (2 * kp + m2 + 1) * 128)
                    out = p[:, m2 * CH:(m2 + 1) * CH]
                    nc.tensor.matmul(out, w1h3[:, :, sl], xh_t[:],
                                     start=True, stop=False, perf_mode=DRM)
                    nc.tensor.matmul(out, w1l3[:, :, sl], xh_t[:],
                                     start=False, stop=False, perf_mode=DRM)
                    nc.tensor.matmul(out, w1h3[:, :, sl], xl_t[:],
                                     start=False, stop=True, perf_mode=DRM)
                sq = sqp.tile([128, 2 * CH], F8, name=f"sq1_{kp}",
                              tag=f"sq1_{kp}")
                nc.scalar.activation(sq[:], p[:], AF.Square, scale=a1)
                hi = rhp.tile([128, 2 * CH], F8, name=f"hi1_{kp}",
                              tag=f"hi1_{kp}")
                nc.vector.tensor_scalar(out=hi[:], in0=p[:], scalar1=0.0,
                                        scalar2=None, op0=ALU.max)
                lo = rlp.tile([128, 2 * CH], F8, name=f"lo1_{kp}",
                              tag=f"lo1_{kp}")
                nc.vector.scalar_tensor_tensor(
                    out=lo[:], in0=p[:], scalar=0.0, in1=hi[:],
                    op0=ALU.max, op1=ALU.subtract)
                his.append(hi); los.append(lo); sqs.append(sq)
            return his, los, sqs

        def chunk_partB(cs, his, los, sqs1):
            st1 = stats_block(sqs1, "1")
            pv1 = sm.tile([1, CH], F32, name="pv1", tag="pv1")
            nc.scalar.activation(pv1[:], st1[0:1, :], AF.Abs_reciprocal_sqrt,
                                 bias=eps_t[:], scale=1.0 / HID)

            hi3 = [h[:].rearrange("p (j c) -> p j c", j=2) for h in his]
            lo3 = [l[:].rearrange("p (j c) -> p j c", j=2) for l in los]
            r2s, sqs2 = [], []
            for kp in range(KP):
                p = ps_mm.tile([128, 2 * CH], F32, name="pmm", tag="pmm")
                for m2 in range(2):
                    sl = slice((2 * kp + m2) * 128, (2 * kp + m2 + 1) * 128)
                    out = p[:, m2 * CH:(m2 + 1) * CH]
                    nk = 3 * KP
                    i = 0
                    for k in range(KP):
                        nc.tensor.matmul(out, w2h4[:, k, :, sl], hi3[k],
                                         start=(i == 0), stop=(i == nk - 1),
                                         perf_mode=DRM); i += 1
                    for k in range(KP):
                        nc.tensor.matmul(out, w2l4[:, k, :, sl], hi3[k],
                                         start=False, stop=(i == nk - 1),
                                         perf_mode=DRM); i += 1
                    for k in range(KP):
                        nc.tensor.matmul(out, w2h4[:, k, :, sl], lo3[k],
                                         start=False, stop=(i == nk - 1),
                                         perf_mode=DRM); i += 1
                sq = sqp.tile([128, 2 * CH], F8, name=f"sq2_{kp}",
                              tag=f"sq2_{kp}")
                nc.scalar.activation(sq[:], p[:], AF.Square, scale=a2)
                r2 = r2p.tile([128, 2 * CH], F16, name=f"r2_{kp}",
                              tag=f"r2_{kp}")
                nc.scalar.activation(r2[:], p[:], AF.Relu)
                r2s.append(r2); sqs2.append(sq)

            st2 = stats_block(sqs2, "2")
            t1 = sm.tile([1, CH], F32, name="t1", tag="t1")
            nc.vector.tensor_mul(t1[:], pv1[:], pv1[:])
            u1 = sm.tile([1, CH], F32, name="u1", tag="u1")
            nc.vector.tensor_mul(u1[:], t1[:], st2[0:1, :])
            pv2 = sm.tile([1, CH], F32, name="pv2", tag="pv2")
            nc.scalar.activation(pv2[:], u1[:], AF.Abs_reciprocal_sqrt,
                                 bias=eps_t[:], scale=1.0 / HID)
            q2 = sm.tile([1, CH], F32, name="q2", tag="q2")
            nc.vector.scalar_tensor_tensor(
                out=q2[:], in0=pv1[:], scalar=qd, in1=pv2[:],
                op0=ALU.mult, op1=ALU.mult)

            p3 = ps_p3.tile([1, CH], F32, name="p3", tag="p3")
            for k in range(MT):
                nc.tensor.matmul(p3[:], w3r[:, k:k + 1],
                                 r2s[k // 2][:, (k % 2) * CH:(k % 2 + 1) * CH],
                                 start=(k == 0), stop=(k == MT - 1))
            osb = sm.tile([1, CH], F32, name="osb", tag="osb")
            nc.vector.tensor_mul(osb[:], p3[:], q2[:])
            if b3f != 0.0:
                nc.vector.tensor_scalar(out=osb[:], in0=osb[:], scalar1=b3f,
                                        scalar2=None, op0=ALU.add)
            nc.sync.dma_start(y.ap()[0:1, cs], osb[:])

        def _cs(c):
            return slice(c * CH, (c + 1) * CH)

        def whole_body():
            if cfg.get("skew", True):
                depth = cfg.get("skew_depth", 1)
                pend = [chunk_partA(_cs(c)) for c in range(min(depth, NCH))]
                for c in range(NCH):
                    if c + depth < NCH:
                        pend.append(chunk_partA(_cs(c + depth)))
                    chunk_partB(_cs(c), *pend.pop(0))
            else:
                for c in range(NCH):
                    chunk_partB(_cs(c), *chunk_partA(_cs(c)))

        if loop_iters is None:
            whole_body()
        else:
            with tc.For_i(0, loop_iters, 1):
                whole_body()
    nc.compile()
    return nc


def _pack_rows(W, groups):
    """[groups*128, N] -> [128, groups*N] with row g*128+p at [p, g*N:...]"""
    W = np.asarray(W)
    n = W.shape[1]
    return np.ascontiguousarray(
        W.reshape(groups, 128, n).transpose(1, 0, 2).reshape(128, groups * n))


def _q8(x):
    import ml_dtypes
    return np.asarray(x, np.float32).astype(ml_dtypes.float8_e4m3)


def make_fast2_maps(inp):
    """Host prep for the fp8 split path. Returns (in_maps, scal)."""
    W1c, bc1, W2c, bc2 = _fold_weights(inp)
    g1 = np.asarray(inp["g1"], np.float64)
    g2 = np.asarray(inp["g2"], np.float64)
    W3 = np.asarray(inp["W3"], np.float32)
    b3 = np.asarray(inp["b3"], np.float32)

    XT = _build_xt_raw(inp)
    Xc = XT[128:185]
    XT[128:185] = np.sign(Xc) * np.log(np.abs(Xc) + 1.0)
    XTF = np.zeros((256, XT.shape[1]), np.float32)
    XTF[0:185] = XT
    XTF[185] = 1.0

    W1g = np.zeros((256, HID))
    W1g[0:185] = W1c.astype(np.float64) * g1[None, :]
    W1g[185] = bc1.astype(np.float64) * g1
    W2g = W2c.astype(np.float64) * g2[None, :]

    xs = (XTF * SX).astype(np.float32)
    xh8 = _q8(xs)
    xl8 = _q8(xs - xh8.astype(np.float32))
    w1s = (W1g * SW1).astype(np.float32)
    w1h8 = _q8(w1s)
    w1l8 = _q8(w1s - w1h8.astype(np.float32))
    w2s = (W2g * SW2).astype(np.float32)
    w2h8 = _q8(w2s)
    w2l8 = _q8(w2s - w2h8.astype(np.float32))

    g1u = float(g1[0])
    g2u = float(g2[0])
    scal = (1.0 / (D1 * g1u), 1.0 / (D2 * g2u), 1.0 / D2, float(b3[0]))
    shared = {
        "w1h": _pack_rows(w1h8, 2), "w1l": _pack_rows(w1l8, 2),
        "w2h": _pack_rows(w2h8, 8), "w2l": _pack_rows(w2l8, 8),
        "w3p": _pack128(W3[:, 0]).astype(np.float16),
    }
    xh8p = _pack_rows(xh8, 2).reshape(128, 2, B)
    xl8p = _pack_rows(xl8, 2).reshape(128, 2, B)
    in_maps = []
    for c in range(N_CORES):
        m = dict(shared)
        sl = slice(c * B_CORE, (c + 1) * B_CORE)
        m["xh"] = np.ascontiguousarray(xh8p[:, :, sl]).reshape(128, 2 * B_CORE)
        m["xl"] = np.ascontiguousarray(xl8p[:, :, sl]).reshape(128, 2 * B_CORE)
        in_maps.append(m)
    return in_maps, scal


def fast2_ok(inp):
    be1 = np.asarray(inp["be1"]); be2 = np.asarray(inp["be2"])
    g1 = np.asarray(inp["g1"]); g2 = np.asarray(inp["g2"])
    _, _, _, bc2 = _fold_weights(inp)
    return (np.all(be1 == 0.0) and np.all(be2 == 0.0)
            and np.all(np.abs(bc2) < 1e-12)
            and np.all(g1 == g1[0]) and np.all(g2 == g2[0])
            and abs(g1[0]) > 1e-6 and abs(g2[0]) > 1e-6)


# ---------------------------------------------------------------- fast3 device
# fp16 matmuls (the empirical PE cost is ~226ns/instruction regardless of
# dtype/K, so fp8 hi-lo splitting loses: it needs 1.5x the instructions)
# with pair-PSUM tiles [128, 2*CH] and pair-wide elementwise ops: half the
# Act/DVE instructions and semaphore traffic of the per-m-tile layout.
FAST3_CFG = dict(xin_bufs=3, r_bufs=2, sq_bufs=2, sm_bufs=3,
                 ps_mm_bufs=3, ps_st_bufs=1, ps_p3_bufs=1,
                 skew=True, skew_depth=1, mm_interleave=1)


def build_fast3(scal, loop_iters=None, cfg=None):
    """scal = (a1, a2, qd, b3f) baked scalar scales (uniform g)."""
    a1, a2, qd, b3f = scal
    cfg = {**FAST3_CFG, **(cfg or {})}
    split = cfg.get("split_psum", False)
    unified = cfg.get("unified_psum", False) or split
    nc = bacc.Bacc("TRN2", target_bir_lowering=False, debug=False)
    xt = nc.dram_tensor("xt", [128, 2 * B_CORE], F16, kind="ExternalInput")
    w1 = nc.dram_tensor("w1", [256, HID], F16, kind="ExternalInput")
    w2 = nc.dram_tensor("w2", [HID, HID], F16, kind="ExternalInput")
    w3p = nc.dram_tensor("w3p", [128, MT], F16, kind="ExternalInput")
    y = nc.dram_tensor("y", [1, B_CORE], F32, kind="ExternalOutput")
    KP = 4

    from contextlib import ExitStack
    with tile.TileContext(nc) as tc, ExitStack() as ctx, \
            nc.allow_low_precision(reason="fp16/fp8 rounding is intentional"):
        const = ctx.enter_context(tc.tile_pool(name="const", bufs=1))
        xin = ctx.enter_context(tc.tile_pool(name="xin", bufs=cfg["xin_bufs"]))
        rp = ctx.enter_context(tc.tile_pool(name="rp", bufs=cfg["r_bufs"]))
        sqp = ctx.enter_context(tc.tile_pool(name="sqp", bufs=cfg["sq_bufs"]))
        sm = ctx.enter_context(tc.tile_pool(name="sm", bufs=cfg["sm_bufs"]))
        ps_mm = ctx.enter_context(
            tc.tile_pool(name="ps_mm",
                         bufs=(3 if split else
                               4 if unified else cfg["ps_mm_bufs"]),
                         space="PSUM"))
        ps_l1 = None
        if split:
            # dedicated L1 psum ring: L1 never waits on L2's consumers,
            # breaking the cross-chunk PE->elementwise->PE->... cycle
            ps_l1 = ctx.enter_context(
                tc.tile_pool(name="ps_l1", bufs=1, space="PSUM"))
        if unified:
            ps_st = ps_p3 = ps_mm
        else:
            ps_st = ctx.enter_context(
                tc.tile_pool(name="ps_st", bufs=cfg["ps_st_bufs"],
                             space="PSUM"))
            ps_p3 = ctx.enter_context(
                tc.tile_pool(name="ps_p3", bufs=cfg["ps_p3_bufs"],
                             space="PSUM"))

        w1a = const.tile([128, HID], F16, tag="w1a")
        nc.sync.dma_start(w1a[:], w1.ap()[0:128, :])
        w1b = const.tile([128, HID], F16, tag="w1b")
        nc.sync.dma_start(w1b[:], w1.ap()[128:256, :])
        w2r = []
        for k in range(MT):
            t = const.tile([128, HID], F16, name=f"w2r{k}", tag=f"w2r{k}")
            nc.sync.dma_start(t[:], w2.ap()[k * 128:(k + 1) * 128, :])
            w2r.append(t)
        w3r = const.tile([128, MT], F16, tag="w3r")
        nc.sync.dma_start(w3r[:], w3p.ap())
        ones_st = const.tile([128, 2, 32], F32, tag="ones_st")
        nc.vector.memset(ones_st[:], 1.0)
        ones8 = const.tile([128, 2, 32], F8, tag="ones8")
        nc.vector.tensor_copy(ones8[:], ones_st[:])
        eps_t = const.tile([1, 1], F32, tag="eps_t")
        nc.vector.memset(eps_t[:], EPS)
        xt3 = xt.ap().rearrange("p (j b) -> p j b", j=2)

        def stats_block(sqs, tag):
            if unified:
                stw_t = ps_mm.tile([128, 2 * CH], F32, name=f"stw{tag}",
                                   tag="pmm")
                stw = stw_t[0:32, 0:CH]
            else:
                stw = ps_st.tile([32, CH], F32, name=f"stw{tag}",
                                 tag="stw")[:]
            for i, sq in enumerate(sqs):
                nc.tensor.matmul(stw, ones8[:],
                                 sq[:].rearrange("p (j c) -> p j c", j=2),
                                 start=(i == 0), stop=(i == len(sqs) - 1),
                                 perf_mode=DRM)
            return stw

        def emit_relu(r_ap, p_ap, kp):
            """Relu PSUM->f16, engine chosen per pair by cfg."""
            from contextlib import nullcontext
            mode = cfg.get("relu_eng", "dve")
            eng = mode if mode in ("dve", "act", "half") else \
                ("act" if kp % 2 else "dve")
            hp = cfg.get("hp_relu", 0)
            with (tc.high_priority(offset=hp) if hp else nullcontext()):
                if eng == "half":
                    nc.vector.tensor_scalar(out=r_ap[:, 0:CH],
                                            in0=p_ap[:, 0:CH], scalar1=0.0,
                                            scalar2=None, op0=ALU.max)
                    nc.scalar.activation(r_ap[:, CH:2 * CH],
                                         p_ap[:, CH:2 * CH], AF.Relu)
                elif eng == "act":
                    nc.scalar.activation(r_ap, p_ap, AF.Relu)
                else:
                    nc.vector.tensor_scalar(out=r_ap, in0=p_ap, scalar1=0.0,
                                            scalar2=None, op0=ALU.max)

        def layer_pairs(w_list, rhs_list, sq_scale, out_tag, out_dt):
            """KP pair-psums, each 2 regions x len(w_list) fp16 chains;
            pair-wide Square->fp8 and Relu->out_dt. Returns (R pair tiles,
            sq pair tiles)."""
            il = cfg.get("mm_interleave", 1)
            rs, sqs = [], []
            for kp0 in range(0, KP, il):
                kps = list(range(kp0, min(kp0 + il, KP)))
                ptiles = [ps_mm.tile([128, 2 * CH], F32, name=f"pm{kp}",
                                     tag="pmm") for kp in kps]
                nk = len(w_list)
                for k in range(nk):
                    for m2 in range(2):
                        for p, kp in zip(ptiles, kps):
                            m = 2 * kp + m2
                            nc.tensor.matmul(
                                p[:, m2 * CH:(m2 + 1) * CH],
                                w_list[k][:, m * 128:(m + 1) * 128],
                                rhs_list[k], start=(k == 0),
                                stop=(k == nk - 1))
                for p, kp in zip(ptiles, kps):
                    sq = sqp.tile([128, 2 * CH], F8, name=f"{out_tag}sq{kp}",
                                  tag=f"{out_tag}sq{kp}")
                    nc.scalar.activation(sq[:], p[:], AF.Square,
                                         scale=sq_scale)
                    r = rp.tile([128, 2 * CH], out_dt, name=f"{out_tag}{kp}",
                                tag=f"{out_tag}{kp}")
                    emit_relu(r[:], p[:], kp)
                    rs.append(r); sqs.append(sq)
            return rs, sqs

        def l1_pair(x_t, kp):
            """One L1 pair-psum: 4 fp16 matmuls + sq + relu."""
            pool = ps_l1 if split else ps_mm
            p = pool.tile([128, 2 * CH], F32, name=f"pa{kp}",
                          tag=("pl1" if split else "pmm"))
            for k in range(2):
                for m2 in range(2):
                    m = 2 * kp + m2
                    nc.tensor.matmul(p[:, m2 * CH:(m2 + 1) * CH],
                                     (w1a, w1b)[k][:, m * 128:(m + 1) * 128],
                                     x_t[:, k, :], start=(k == 0),
                                     stop=(k == 1))
            sq = sqp.tile([128, 2 * CH], F8, name=f"R1sq{kp}",
                          tag=f"R1sq{kp}")
            if cfg.get("swap_eng", False):
                nc.vector.tensor_mul(sq[:], p[:], p[:])
                r = rp.tile([128, 2 * CH], F16, name=f"R1{kp}",
                            tag=f"R1{kp}")
                nc.scalar.activation(r[:], p[:], AF.Relu)
            else:
                nc.scalar.activation(sq[:], p[:], AF.Square, scale=a1)
                r = rp.tile([128, 2 * CH], F16, name=f"R1{kp}",
                            tag=f"R1{kp}")
                emit_relu(r[:], p[:], kp)
            if cfg.get("dma_launder", False):
                rd = rp.tile([128, 2 * CH], F16, name=f"R1d{kp}",
                             tag=f"R1d{kp}")
                nc.sync.dma_start(rd[:], r[:])
                return rd, sq
            return r, sq

        def chunk_partA(cs):
            x_t = xin.tile([128, 2, CH], F16, name="x_t", tag="x_t")
            nc.sync.dma_start(x_t[:], xt3[:, :, cs])
            R1, sq1 = [], []
            for kp in range(KP):
                r, sq = l1_pair(x_t, kp)
                R1.append(r); sq1.append(sq)
            return R1, sq1

        def chunk_partA_dma(cs):
            x_t = xin.tile([128, 2, CH], F16, name="x_t", tag="x_t")
            nc.sync.dma_start(x_t[:], xt3[:, :, cs])
            return x_t

        # diagnostic: constant rhs tiles to cut matmul->elementwise deps
        if cfg.get("dep_cut", False):
            cst = const.tile([128, 2 * CH], F16, tag="cst")
            nc.vector.memset(cst[:], 0.01)
            cst8 = const.tile([128, 2 * CH], F8, tag="cst8")
            nc.vector.memset(cst8[:], 0.01)

        def chunk_partB(cs, R1, sq1, next_x=None, next_out=None):
            if cfg.get("dep_cut", False):
                R1 = [cst] * KP
                sq1 = [cst8] * KP
            st1 = stats_block(sq1, "1")
            pv1 = sm.tile([1, CH], F32, name="pv1", tag="pv1")
            nc.scalar.activation(pv1[:], st1[0:1, :], AF.Abs_reciprocal_sqrt,
                                 bias=eps_t[:], scale=1.0 / HID)

            r1sl = [R1[k // 2][:, (k % 2) * CH:(k % 2 + 1) * CH]
                    for k in range(MT)]
            if next_x is None:
                R2, sq2 = layer_pairs(w2r, r1sl, a2, "R2", F16)
            else:
                # software-pipeline: slot next chunk's L1 pairs between this
                # chunk's L2 pairs so PE never bursts ahead of Act/DVE.
                R2, sq2 = [], []
                for kp in range(KP):
                    p = ps_mm.tile([128, 2 * CH], F32, name=f"pb{kp}",
                                   tag="pmm")
                    for k in range(MT):
                        for m2 in range(2):
                            m = 2 * kp + m2
                            nc.tensor.matmul(
                                p[:, m2 * CH:(m2 + 1) * CH],
                                w2r[k][:, m * 128:(m + 1) * 128],
                                r1sl[k], start=(k == 0), stop=(k == MT - 1))
                    sq = sqp.tile([128, 2 * CH], F8, name=f"R2sq{kp}",
                                  tag=f"R2sq{kp}")
                    nc.scalar.activation(sq[:], p[:], AF.Square, scale=a2)
                    r = rp.tile([128, 2 * CH], F16, name=f"R2{kp}",
                                tag=f"R2{kp}")
                    emit_relu(r[:], p[:], kp)
                    R2.append(r); sq2.append(sq)
                    ra, sqa = l1_pair(next_x, kp)
                    next_out[0].append(ra); next_out[1].append(sqa)

            if cfg.get("dep_cut", False):
                R2 = [cst] * KP
                sq2 = [cst8] * KP
            st2 = stats_block(sq2, "2")
            t1 = sm.tile([1, CH], F32, name="t1", tag="t1")
            nc.vector.tensor_mul(t1[:], pv1[:], pv1[:])
            u1 = sm.tile([1, CH], F32, name="u1", tag="u1")
            nc.vector.tensor_mul(u1[:], t1[:], st2[0:1, :])
            pv2 = sm.tile([1, CH], F32, name="pv2", tag="pv2")
            nc.scalar.activation(pv2[:], u1[:], AF.Abs_reciprocal_sqrt,
                                 bias=eps_t[:], scale=1.0 / HID)
            q2 = sm.tile([1, CH], F32, name="q2", tag="q2")
            nc.vector.scalar_tensor_tensor(
                out=q2[:], in0=pv1[:], scalar=qd, in1=pv2[:],
                op0=ALU.mult, op1=ALU.mult)

            p3 = ps_p3.tile([1, CH], F32, name="p3", tag="p3")
            for k in range(MT):
                nc.tensor.matmul(p3[:], w3r[:, k:k + 1],
                                 R2[k // 2][:, (k % 2) * CH:(k % 2 + 1) * CH],
                                 start=(k == 0), stop=(k == MT - 1))
            osb = sm.tile([1, CH], F32, name="osb", tag="osb")
            nc.vector.tensor_mul(osb[:], p3[:], q2[:])
            if b3f != 0.0:
                nc.vector.tensor_scalar(out=osb[:], in0=osb[:], scalar1=b3f,
                                        scalar2=None, op0=ALU.add)
            nc.sync.dma_start(y.ap()[0:1, cs], osb[:])

        def _cs3(c):
            return slice(c * CH, (c + 1) * CH)

        def l2_pair(kp, r1sl):
            """One L2 pair-psum: 16 fp16 matmuls + sq2 + relu2."""
            p = ps_mm.tile([128, 2 * CH], F32, name=f"pb{kp}", tag="pmm")
            for k in range(MT):
                for m2 in range(2):
                    m = 2 * kp + m2
                    nc.tensor.matmul(p[:, m2 * CH:(m2 + 1) * CH],
                                     w2r[k][:, m * 128:(m + 1) * 128],
                                     r1sl[k], start=(k == 0),
                                     stop=(k == MT - 1))
            sq = sqp.tile([128, 2 * CH], F8, name=f"R2sq{kp}",
                          tag=f"R2sq{kp}")
            if cfg.get("swap_eng", False):
                nc.vector.tensor_mul(sq[:], p[:], p[:])
                r = rp.tile([128, 2 * CH], F16, name=f"R2{kp}",
                            tag=f"R2{kp}")
                nc.scalar.activation(r[:], p[:], AF.Relu)
            else:
                nc.scalar.activation(sq[:], p[:], AF.Square, scale=a2)
                r = rp.tile([128, 2 * CH], F16, name=f"R2{kp}",
                            tag=f"R2{kp}")
                emit_relu(r[:], p[:], kp)
            if cfg.get("dma_launder", False):
                rd = rp.tile([128, 2 * CH], F16, name=f"R2d{kp}",
                             tag=f"R2d{kp}")
                nc.sync.dma_start(rd[:], r[:])
                return rd, sq
            return r, sq

        def sched2_body():
            """Latency-aware PE order: stats/L3 placed where inputs are
            already computed; next-chunk L1 pairs used as PE filler."""
            x_cur = chunk_partA_dma(_cs3(0))
            R1 = []
            sq1 = []
            for kp in range(KP):
                r, sq = l1_pair(x_cur, kp)
                R1.append(r); sq1.append(sq)
            dc_relu = cfg.get("dep_cut_relu", False)
            dc4 = cfg.get("dep_cut_relu4", False)
            dc_sq = cfg.get("dep_cut_sq", False)
            if dc_relu or dc_sq or dc4:
                cst4 = []
                for i in range(KP):
                    t = const.tile([128, 2 * CH], F16, name=f"cst2_{i}",
                                   tag=f"cst2_{i}")
                    nc.vector.memset(t[:], 0.01)
                    cst4.append(t)
                cst2 = cst4[0]
                cst28 = const.tile([128, 2 * CH], F8, tag="cst28")
                nc.vector.memset(cst28[:], 0.01)
            for c in range(NCH):
                x_nxt = chunk_partA_dma(_cs3(c + 1)) if c + 1 < NCH else None
                if dc_relu:
                    R1 = [cst2] * KP
                elif dc4:
                    R1 = list(cst4)
                if dc_sq:
                    sq1 = [cst28] * KP
                r1sl = [R1[k // 2][:, (k % 2) * CH:(k % 2 + 1) * CH]
                        for k in range(MT)]
                R2, sq2 = [], []
                nR1, nsq1 = [], []

                def maybe_l1(kp):
                    # split mode: spread next-chunk L1 pairs one after each
                    # L2 pair so the 1-buf L1 ring never stalls
                    if split and x_nxt is not None:
                        r, sq = l1_pair(x_nxt, kp)
                        nR1.append(r); nsq1.append(sq)

                for kp in (0, 1):
                    r, sq = l2_pair(kp, r1sl)
                    R2.append(r); sq2.append(sq)
                    maybe_l1(kp)
                st1 = stats_block(sq1, "1")
                pv1 = sm.tile([1, CH], F32, name="pv1", tag="pv1")
                nc.scalar.activation(pv1[:], st1[0:1, :],
                                     AF.Abs_reciprocal_sqrt,
                                     bias=eps_t[:], scale=1.0 / HID)
                for kp in (2, 3):
                    r, sq = l2_pair(kp, r1sl)
                    R2.append(r); sq2.append(sq)
                    maybe_l1(kp)
                if not split and x_nxt is not None:
                    for kp in (0, 1):
                        r, sq = l1_pair(x_nxt, kp)
                        nR1.append(r); nsq1.append(sq)
                if dc_relu:
                    R2 = [cst2] * KP
                elif dc4:
                    R2 = list(cst4)
                if dc_sq:
                    sq2 = [cst28] * KP
                st2 = stats_block(sq2, "2")
                t1 = sm.tile([1, CH], F32, name="t1", tag="t1")
                nc.vector.tensor_mul(t1[:], pv1[:], pv1[:])
                u1 = sm.tile([1, CH], F32, name="u1", tag="u1")
                nc.vector.tensor_mul(u1[:], t1[:], st2[0:1, :])
                pv2 = sm.tile([1, CH], F32, name="pv2", tag="pv2")
                nc.scalar.activation(pv2[:], u1[:], AF.Abs_reciprocal_sqrt,
                                     bias=eps_t[:], scale=1.0 / HID)
                q2 = sm.tile([1, CH], F32, name="q2", tag="q2")
                nc.vector.scalar_tensor_tensor(
                    out=q2[:], in0=pv1[:], scalar=qd, in1=pv2[:],
                    op0=ALU.mult, op1=ALU.mult)
                if unified:
                    p3t = ps_mm.tile([128, 2 * CH], F32, name="p3t",
                                     tag="pmm")
                    p3a = p3t[0:1, 0:CH]
                else:
                    p3a = ps_p3.tile([1, CH], F32, name="p3", tag="p3")[:]
                for k in range(MT):
                    nc.tensor.matmul(
                        p3a, w3r[:, k:k + 1],
                        R2[k // 2][:, (k % 2) * CH:(k % 2 + 1) * CH],
                        start=(k == 0), stop=(k == MT - 1))
                osb = sm.tile([1, CH], F32, name="osb", tag="osb")
                nc.vector.tensor_mul(osb[:], p3a, q2[:])
                if b3f != 0.0:
                    nc.vector.tensor_scalar(out=osb[:], in0=osb[:],
                                            scalar1=b3f, scalar2=None,
                                            op0=ALU.add)
                nc.sync.dma_start(y.ap()[0:1, _cs3(c)], osb[:])
                if not split and x_nxt is not None:
                    for kp in (2, 3):
                        r, sq = l1_pair(x_nxt, kp)
                        nR1.append(r); nsq1.append(sq)
                R1, sq1 = nR1, nsq1

        def whole_body():
            if cfg.get("sched2", True):
                sched2_body()
            elif cfg.get("pipe", False):
                # fine-grained software pipeline across chunks
                cur = chunk_partA(_cs3(0))
                for c in range(NCH):
                    if c + 1 < NCH:
                        nxt_x = chunk_partA_dma(_cs3(c + 1))
                        nxt = ([], [])
                        chunk_partB(_cs3(c), *cur, next_x=nxt_x,
                                    next_out=nxt)
                        cur = nxt
                    else:
                        chunk_partB(_cs3(c), *cur)
            elif cfg.get("skew", True):
                depth = cfg.get("skew_depth", 1)
                pend = [chunk_partA(_cs3(c)) for c in range(min(depth, NCH))]
                for c in range(NCH):
                    if c + depth < NCH:
                        pend.append(chunk_partA(_cs3(c + depth)))
                    chunk_partB(_cs3(c), *pend.pop(0))
            else:
                for c in range(NCH):
                    chunk_partB(_cs3(c), *chunk_partA(_cs3(c)))

        reps = cfg.get("unroll_reps")
        if reps:
            for _ in range(reps):
                whole_body()
        elif loop_iters is None:
            whole_body()
        else:
            with tc.For_i(0, loop_iters, 1):
                whole_body()
    nc.compile()
    return nc


def make_fast3_maps(inp):
    """Host prep for fast3. Returns (in_maps, scal)."""
    W1c, bc1, W2c, bc2 = _fold_weights(inp)
    g1 = np.asarray(inp["g1"], np.float64)
    g2 = np.asarray(inp["g2"], np.float64)
    W3 = np.asarray(inp["W3"], np.float32)
    b3 = np.asarray(inp["b3"], np.float32)

    XT = _build_xt_raw(inp)
    Xc = XT[128:185]
    XT[128:185] = np.sign(Xc) * np.log(np.abs(Xc) + 1.0)
    XTF = np.zeros((256, XT.shape[1]), np.float32)
    XTF[0:185] = XT
    XTF[185] = 1.0

    W1g = np.zeros((256, HID))
    W1g[0:185] = W1c.astype(np.float64) * g1[None, :]
    W1g[185] = bc1.astype(np.float64) * g1
    W2g = (W2c.astype(np.float64) * g2[None, :]).astype(np.float16)

    g1u = float(g1[0])
    g2u = float(g2[0])
    scal = (1.0 / g1u, 1.0 / g2u, 1.0, float(b3[0]))
    shared = {
        "w1": W1g.astype(np.float16), "w2": W2g,
        "w3p": _pack128(W3[:, 0]).astype(np.float16),
    }
    xp = _pack_rows(XTF.astype(np.float16), 2).reshape(128, 2, B)
    in_maps = []
    for c in range(N_CORES):
        m = dict(shared)
        sl = slice(c * B_CORE, (c + 1) * B_CORE)
        m["xt"] = np.ascontiguousarray(xp[:, :, sl]).reshape(128, 2 * B_CORE)
        in_maps.append(m)
    return in_maps, scal


# ---------------------------------------------------------------- fast4 device
# CH=1024 chunks (each m-tile psum spans 2 banks, written by 2 matmul
# half-chains). Halves the number of chunks and therefore the number of
# cross-engine PE waits, which cost ~1.5us each on HW regardless of slack.
# Contraction chains start at the newest-written rhs tile so a single
# watermark wait covers all eight.
CH4 = 1024
NCH4 = B_CORE // CH4


def build_fast4(scal, loop_iters=None, cfg=None):
    a1, a2, qd, b3f = scal
    cfg = cfg or {}
    nc = bacc.Bacc("TRN2", target_bir_lowering=False, debug=False)
    xt = nc.dram_tensor("xt", [128, 2 * B_CORE], F16, kind="ExternalInput")
    w1 = nc.dram_tensor("w1", [256, HID], F16, kind="ExternalInput")
    w2 = nc.dram_tensor("w2", [HID, HID], F16, kind="ExternalInput")
    w3p = nc.dram_tensor("w3p", [128, MT], F16, kind="ExternalInput")
    y = nc.dram_tensor("y", [1, B_CORE], F32, kind="ExternalOutput")

    from contextlib import ExitStack
    with tile.TileContext(nc) as tc, ExitStack() as ctx, \
            nc.allow_low_precision(reason="fp16/fp8 rounding is intentional"):
        const = ctx.enter_context(tc.tile_pool(name="const", bufs=1))
        xin = ctx.enter_context(tc.tile_pool(name="xin", bufs=3))
        rp = ctx.enter_context(tc.tile_pool(name="rp", bufs=2))
        sqp = ctx.enter_context(tc.tile_pool(name="sqp", bufs=2))
        sm = ctx.enter_context(tc.tile_pool(name="sm", bufs=4))
        ps_mm = ctx.enter_context(
            tc.tile_pool(name="ps_mm", bufs=3, space="PSUM"))
        ps_st = ctx.enter_context(
            tc.tile_pool(name="ps_st", bufs=1, space="PSUM"))
        ps_p3 = ctx.enter_context(
            tc.tile_pool(name="ps_p3", bufs=1, space="PSUM"))

        w1a = const.tile([128, HID], F16, tag="w1a")
        nc.sync.dma_start(w1a[:], w1.ap()[0:128, :])
        w1b = const.tile([128, HID], F16, tag="w1b")
        nc.sync.dma_start(w1b[:], w1.ap()[128:256, :])
        w2r = []
        for k in range(MT):
            t = const.tile([128, HID], F16, name=f"w2r{k}", tag=f"w2r{k}")
            nc.sync.dma_start(t[:], w2.ap()[k * 128:(k + 1) * 128, :])
            w2r.append(t)
        w3r = const.tile([128, MT], F16, tag="w3r")
        nc.sync.dma_start(w3r[:], w3p.ap())
        ones_st = const.tile([128, 2, 32], F32, tag="ones_st")
        nc.vector.memset(ones_st[:], 1.0)
        ones8 = const.tile([128, 2, 32], F8, tag="ones8")
        nc.vector.tensor_copy(ones8[:], ones_st[:])
        eps_t = const.tile([1, 1], F32, tag="eps_t")
        nc.vector.memset(eps_t[:], EPS)
        xt3 = xt.ap().rearrange("p (j b) -> p j b", j=2)

        def stats4(sqs, h, tag):
            """[32, 512] DRM stats over 4 sq-pair tiles, half h.
            Chain starts at the newest pair (index 3)."""
            stw = ps_st.tile([32, 512], F32, name=f"stw{tag}{h}", tag="stw")
            order = [3, 0, 1, 2]
            for i, kp in enumerate(order):
                nc.tensor.matmul(stw[:], ones8[:],
                                 sqs[kp][:, :, h * 512:(h + 1) * 512],
                                 start=(i == 0), stop=(i == 3),
                                 perf_mode=DRM)
            return stw

        def pv_of(st, tag):
            pv = sm.tile([1, 512], F32, name=f"pv{tag}", tag=f"pv{tag}")
            nc.scalar.activation(pv[:], st[0:1, :], AF.Abs_reciprocal_sqrt,
                                 bias=eps_t[:], scale=1.0 / HID)
            return pv

        def l1_m(x_t, m, sqt):
            p = ps_mm.tile([128, CH4], F32, name=f"pa{m}", tag="pmm")
            for h in range(2):
                for k in range(2):
                    nc.tensor.matmul(
                        p[:, h * 512 + 0:h * 512 + 512],
                        (w1a, w1b)[k][:, m * 128:(m + 1) * 128],
                        x_t[:, k, h * 512:(h + 1) * 512],
                        start=(k == 0), stop=(k == 1))
            nc.scalar.activation(sqt[m // 2][:, m % 2, :], p[:], AF.Square,
                                 scale=a1)
            r = rp.tile([128, CH4], F16, name=f"R1_{m}", tag=f"R1_{m}")
            nc.vector.tensor_scalar(out=r[:], in0=p[:], scalar1=0.0,
                                    scalar2=None, op0=ALU.max)
            return r

        def partA(c):
            x_t = xin.tile([128, 2, CH4], F16, name="x_t", tag="x_t")
            nc.sync.dma_start(x_t[:], xt3[:, :, c * CH4:(c + 1) * CH4])
            sqt = [sqp.tile([128, 2, CH4], F8, name=f"sq1_{i}",
                            tag=f"sq1_{i}") for i in range(4)]
            R1 = [l1_m(x_t, m, sqt) for m in range(MT)]
            return R1, sqt

        # k-order for L2/L3 chains: newest rhs tile first
        KORD = [7, 0, 1, 2, 3, 4, 5, 6]

        def partB(c, R1, sq1):
            st1 = [stats4(sq1, h, "1") for h in range(2)]
            pv1 = [pv_of(st1[h], f"1{h}") for h in range(2)]

            sq2t = [sqp.tile([128, 2, CH4], F8, name=f"sq2_{i}",
                             tag=f"sq2_{i}") for i in range(4)]
            R2 = []
            for m in range(MT):
                p = ps_mm.tile([128, CH4], F32, name=f"pb{m}", tag="pmm")
                for h in range(2):
                    for i, k in enumerate(KORD):
                        nc.tensor.matmul(
                            p[:, h * 512:h * 512 + 512],
                            w2r[k][:, m * 128:(m + 1) * 128],
                            R1[k][:, h * 512:(h + 1) * 512],
                            start=(i == 0), stop=(i == MT - 1))
                nc.scalar.activation(sq2t[m // 2][:, m % 2, :], p[:],
                                     AF.Square, scale=a2)
                r = rp.tile([128, CH4], F16, name=f"R2_{m}", tag=f"R2_{m}")
                nc.vector.tensor_scalar(out=r[:], in0=p[:], scalar1=0.0,
                                        scalar2=None, op0=ALU.max)
                R2.append(r)

            osb = sm.tile([1, CH4], F32, name="osb", tag="osb")
            for h in range(2):
                st2 = stats4(sq2t, h, "2")
                t1 = sm.tile([1, 512], F32, name=f"t1{h}", tag="t1")
                nc.vector.tensor_mul(t1[:], pv1[h][:], pv1[h][:])
                u1 = sm.tile([1, 512], F32, name=f"u1{h}", tag="u1")
                nc.vector.tensor_mul(u1[:], t1[:], st2[0:1, :])
                pv2 = pv_of(u1, f"2{h}")
                q2 = sm.tile([1, 512], F32, name=f"q2{h}", tag="q2")
                nc.vector.scalar_tensor_tensor(
                    out=q2[:], in0=pv1[h][:], scalar=qd, in1=pv2[:],
                    op0=ALU.mult, op1=ALU.mult)
                p3 = ps_p3.tile([1, 512], F32, name=f"p3{h}", tag="p3")
                for i, k in enumerate(KORD):
                    nc.tensor.matmul(p3[:], w3r[:, k:k + 1],
                                     R2[k][:, h * 512:(h + 1) * 512],
                                     start=(i == 0), stop=(i == MT - 1))
                ob = osb[:, h * 512:(h + 1) * 512]
                nc.vector.tensor_mul(ob, p3[:], q2[:])
                if b3f != 0.0:
                    nc.vector.tensor_scalar(out=ob, in0=ob, scalar1=b3f,
                                            scalar2=None, op0=ALU.add)
            nc.sync.dma_start(y.ap()[0:1, c * CH4:(c + 1) * CH4], osb[:])

        def whole_body():
            pend = partA(0)
            for c in range(NCH4):
                if c + 1 < NCH4:
                    nxt = partA(c + 1)
                else:
                    nxt = None
                partB(c, *pend)
                pend = nxt

        if loop_iters is None:
            whole_body()
        else:
            with tc.For_i(0, loop_iters, 1):
                whole_body()
    nc.compile()
    return nc


# ---------------------------------------------------------------- fast device
FAST_CFG = dict(xin_bufs=3, r_bufs=2, sq_bufs=2, sm_bufs=3,
                ps_mm_bufs=3, ps_st_bufs=2, ps_p3_bufs=2,
                relu_split=0, stats_late=True)


def build_fast(has_b3, loop_iters=None, cfg=None):
    """Fast-path program. has_b3: include final bias add."""
    cfg = {**FAST_CFG, **(cfg or {})}
    nc = bacc.Bacc("TRN2", target_bir_lowering=False, debug=False)
    xt = nc.dram_tensor("xt", [256, B_CORE], F16, kind="ExternalInput")
    w1 = nc.dram_tensor("w1", [256, HID], F16, kind="ExternalInput")
    w2 = nc.dram_tensor("w2", [HID, HID], F16, kind="ExternalInput")
    w3p = nc.dram_tensor("w3p", [128, MT], F16, kind="ExternalInput")
    s1p = nc.dram_tensor("s1p", [128, MT], F32, kind="ExternalInput")
    s2p = nc.dram_tensor("s2p", [128, MT], F32, kind="ExternalInput")
    b3t = nc.dram_tensor("b3t", [1, 1], F32, kind="ExternalInput")
    y = nc.dram_tensor("y", [1, B_CORE], F32, kind="ExternalOutput")

    from contextlib import ExitStack
    with tile.TileContext(nc) as tc, ExitStack() as ctx, \
            nc.allow_low_precision(reason="fp16/fp8 rounding is intentional"):
        const = ctx.enter_context(tc.tile_pool(name="const", bufs=1))
        xin = ctx.enter_context(tc.tile_pool(name="xin", bufs=cfg["xin_bufs"]))
        rp = ctx.enter_context(tc.tile_pool(name="rp", bufs=cfg["r_bufs"]))
        sqp = ctx.enter_context(tc.tile_pool(name="sqp", bufs=cfg["sq_bufs"]))
        sm = ctx.enter_context(tc.tile_pool(name="sm", bufs=cfg["sm_bufs"]))
        ps_mm = ctx.enter_context(
            tc.tile_pool(name="ps_mm", bufs=cfg["ps_mm_bufs"], space="PSUM"))
        ps_st = ctx.enter_context(
            tc.tile_pool(name="ps_st", bufs=cfg["ps_st_bufs"], space="PSUM"))
        ps_p3 = ctx.enter_context(
            tc.tile_pool(name="ps_p3", bufs=cfg["ps_p3_bufs"], space="PSUM"))

        # ---- one-time constants
        w1a = const.tile([128, HID], F16, tag="w1a")
        nc.sync.dma_start(w1a[:], w1.ap()[0:128, :])
        w1b = const.tile([128, HID], F16, tag="w1b")
        nc.sync.dma_start(w1b[:], w1.ap()[128:256, :])
        w2r = []
        for k in range(MT):
            t = const.tile([128, HID], F16, name=f"w2r{k}", tag=f"w2r{k}")
            nc.sync.dma_start(t[:], w2.ap()[k * 128:(k + 1) * 128, :])
            w2r.append(t)
        w3r = const.tile([128, MT], F16, tag="w3r")
        nc.sync.dma_start(w3r[:], w3p.ap())
        s1 = const.tile([128, MT], F32, tag="s1")
        nc.sync.dma_start(s1[:], s1p.ap())
        s2 = const.tile([128, MT], F32, tag="s2")
        nc.sync.dma_start(s2[:], s2p.ap())
        b3s = const.tile([1, 1], F32, tag="b3s")
        nc.sync.dma_start(b3s[:], b3t.ap())
        ones_st = const.tile([128, 2, 32], F32, tag="ones_st")
        nc.vector.memset(ones_st[:], 1.0)
        ones8 = const.tile([128, 2, 32], F8, tag="ones8")
        nc.vector.tensor_copy(ones8[:], ones_st[:])
        eps_t = const.tile([1, 1], F32, tag="eps_t")
        nc.vector.memset(eps_t[:], EPS)

        r_split = cfg.get("r_split", False)

        def layer_block(G_pool, w_tiles, rhs_list, sq_s, out_tag):
            """Emit MT m-tiles: matmuls + Square->fp8 pairs + relu->f16.
            Returns (R slices list of [128, CH] f16, sq pair tiles list)."""
            if r_split:
                Rs = [rp.tile([128, CH], F16, name=f"{out_tag}_{m}",
                              tag=f"{out_tag}_{m}") for m in range(MT)]
                rsl = [t[:] for t in Rs]
            else:
                R = rp.tile([128, MT * CH], F16, name=out_tag, tag=out_tag)
                rsl = [R[:, m * CH:(m + 1) * CH] for m in range(MT)]
            sqs = []
            for pr in range(MT // 2):
                sq = sqp.tile([128, 2, CH], F8, name=f"{out_tag}sq{pr}",
                              tag=f"{out_tag}sq{pr}")
                sqs.append(sq)
            il = cfg.get("mm_interleave", 1)
            nk = len(w_tiles)
            for m0 in range(0, MT, il):
                ms = list(range(m0, min(m0 + il, MT)))
                ptiles = [G_pool.tile([128, CH], F32, name=f"pmm{m}",
                                      tag="pmm") for m in ms]
                for k in range(nk):
                    for p, m in zip(ptiles, ms):
                        nc.tensor.matmul(
                            p[:], w_tiles[k][:, m * 128:(m + 1) * 128],
                            rhs_list[k], start=(k == 0), stop=(k == nk - 1))
                for p, m in zip(ptiles, ms):
                    nc.scalar.activation(sqs[m // 2][:, m % 2, :], p[:],
                                         AF.Square, scale=sq_s[:, m:m + 1])
                    nc.vector.tensor_scalar(out=rsl[m], in0=p[:], scalar1=0.0,
                                            scalar2=None, op0=ALU.max)
            return rsl, sqs

        def stats_block(sqs, tag):
            from contextlib import nullcontext
            off = cfg.get("stats_prio_off", 0)
            stw = ps_st.tile([32, CH], F32, name=f"stw{tag}", tag="stw")
            with (tc.high_priority(offset=off) if off else nullcontext()):
                for i, sq in enumerate(sqs):
                    nc.tensor.matmul(stw[:], ones8[:], sq[:], start=(i == 0),
                                     stop=(i == len(sqs) - 1), perf_mode=DRM)
            return stw

        def chunk_partA(cs):
            """DMA + layer 1 + LN1 elementwise (PE work available early)."""
            xa = xin.tile([128, CH], F16, name="xa", tag="xa")
            nc.sync.dma_start(xa[:], xt.ap()[0:128, cs])
            xb = xin.tile([128, CH], F16, name="xb", tag="xb")
            nc.sync.dma_start(xb[:], xt.ap()[128:256, cs])
            R1, sq1 = layer_block(ps_mm, [w1a, w1b], [xa[:], xb[:]], s1, "R1")
            return R1, sq1

        def chunk_partB(cs, R1, sq1):
            """stats1, layer 2, LN2, layer 3, output."""
            st1 = stats_block(sq1, "1")
            pv1 = sm.tile([1, CH], F32, name="pv1", tag="pv1")
            nc.scalar.activation(pv1[:], st1[0:1, :], AF.Abs_reciprocal_sqrt,
                                 bias=eps_t[:], scale=1.0 / HID)

            R2, sq2 = layer_block(ps_mm, w2r, list(R1), s2, "R2")
            st2 = stats_block(sq2, "2")
            t1 = sm.tile([1, CH], F32, name="t1", tag="t1")
            nc.vector.tensor_mul(t1[:], pv1[:], pv1[:])
            u1 = sm.tile([1, CH], F32, name="u1", tag="u1")
            nc.vector.tensor_mul(u1[:], t1[:], st2[0:1, :])
            pv2 = sm.tile([1, CH], F32, name="pv2", tag="pv2")
            nc.scalar.activation(pv2[:], u1[:], AF.Abs_reciprocal_sqrt,
                                 bias=eps_t[:], scale=1.0 / HID)
            q2 = sm.tile([1, CH], F32, name="q2", tag="q2")
            nc.vector.tensor_mul(q2[:], pv1[:], pv2[:])

            p3 = ps_p3.tile([1, CH], F32, name="p3", tag="p3")
            for k in range(MT):
                nc.tensor.matmul(p3[:], w3r[:, k:k + 1], R2[k],
                                 start=(k == 0), stop=(k == MT - 1))
            osb = sm.tile([1, CH], F32, name="osb", tag="osb")
            nc.vector.tensor_mul(osb[:], p3[:], q2[:])
            if has_b3:
                b3b = bass.AP(tensor=b3s[:].tensor, offset=b3s[:].offset,
                              ap=[b3s[:].ap[0], [0, CH]])
                nc.vector.tensor_tensor(out=osb[:], in0=osb[:], in1=b3b,
                                        op=ALU.add)
            nc.sync.dma_start(y.ap()[0:1, cs], osb[:])

        def _cs(c):
            return slice(c * CH, (c + 1) * CH)

        def whole_body():
            cl = cfg.get("chunk_loop")
            if cl is not None:
                unroll = cfg.get("chunk_unroll", 1)
                hint = ((mybir.EngineType.PE,)
                        if cfg.get("hint_pe", False) else ())
                stag = cfg.get("staggered_reset", False)
                with tc.For_i(0, NCH // unroll, 1, hint_engines=hint,
                              staggered_reset=stag) as iv:
                    for u in range(unroll):
                        cs = bass.ds(iv * (CH * unroll) + u * CH, CH)
                        chunk_partB(cs, *chunk_partA(cs))
            elif cfg.get("skew", True):
                depth = cfg.get("skew_depth", 1)
                pend = [chunk_partA(_cs(c)) for c in range(min(depth, NCH))]
                for c in range(NCH):
                    if c + depth < NCH:
                        pend.append(chunk_partA(_cs(c + depth)))
                    chunk_partB(_cs(c), *pend.pop(0))
            else:
                for c in range(NCH):
                    chunk_partB(_cs(c), *chunk_partA(_cs(c)))

        if loop_iters is None:
            whole_body()
        else:
            with tc.For_i(0, loop_iters, 1):
                whole_body()
    nc.compile()
    return nc


def make_fast_maps(inp):
    """Host prep for the fast path. Returns (in_maps, has_b3)."""
    W1c, bc1, W2c, bc2 = _fold_weights(inp)
    g1 = np.asarray(inp["g1"], np.float64)
    g2 = np.asarray(inp["g2"], np.float64)
    W3 = np.asarray(inp["W3"], np.float32)
    b3 = np.asarray(inp["b3"], np.float32)

    XT = _build_xt_raw(inp)
    Xc = XT[128:185]
    XT[128:185] = np.sign(Xc) * np.log(np.abs(Xc) + 1.0)
    XTF = np.zeros((256, XT.shape[1]), np.float16)
    XTF[0:185] = XT.astype(np.float16)
    XTF[185] = 1.0

    W1g = (W1c.astype(np.float64) * g1[None, :])
    bc1g = bc1.astype(np.float64) * g1
    W1full = np.zeros((256, HID), np.float16)
    W1full[0:185] = W1g.astype(np.float16)
    W1full[185] = bc1g.astype(np.float16)
    W2g = (W2c.astype(np.float64) * g2[None, :]).astype(np.float16)

    shared = {
        "w1": W1full, "w2": W2g,
        "w3p": _pack128(W3[:, 0]).astype(np.float16),
        "s1p": _pack128(1.0 / g1), "s2p": _pack128(1.0 / g2),
        "b3t": b3.reshape(1, 1).astype(np.float32),
    }
    in_maps = []
    for c in range(N_CORES):
        m = dict(shared)
        m["xt"] = np.ascontiguousarray(XTF[:, c * B_CORE:(c + 1) * B_CORE])
        in_maps.append(m)
    return in_maps, bool(np.any(b3 != 0.0))


def fast_path_ok(inp):
    be1 = np.asarray(inp["be1"]); be2 = np.asarray(inp["be2"])
    g1 = np.asarray(inp["g1"]); g2 = np.asarray(inp["g2"])
    _, _, _, bc2 = _fold_weights(inp)
    return (np.all(be1 == 0.0) and np.all(be2 == 0.0)
            and np.all(np.abs(bc2) < 1e-12)
            and np.all(np.abs(g1) > 1e-6) and np.all(np.abs(g2) > 1e-6))


# ---------------------------------------------------------------- legacy path
DEFAULT_CFG = dict(h_bufs=1, sq_bufs=1, r1_bufs=1, r2_bufs=1,
                   ps_mm_bufs=3, xin_bufs=3, xr_bufs=2, per_m=False,
                   l2_fp16=False, h_fp16=False)


def build_program(simple_affine, loop_iters=None, cfg=None):
    """Legacy general-path program (arbitrary affine/bias)."""
    cfg = {**DEFAULT_CFG, **(cfg or {})}
    nc = bacc.Bacc("TRN2", target_bir_lowering=False, debug=False)
    xt = nc.dram_tensor("xt", [KA + KC, B_CORE], F32, kind="ExternalInput")
    w1 = nc.dram_tensor("w1", [KA + KC, HID], F32, kind="ExternalInput")
    w2 = nc.dram_tensor("w2", [HID, HID], F32, kind="ExternalInput")
    w3p = nc.dram_tensor("w3p", [128, MT], F32, kind="ExternalInput")
    bc1p = nc.dram_tensor("bc1p", [128, MT], F32, kind="ExternalInput")
    bc2p = nc.dram_tensor("bc2p", [128, MT], F32, kind="ExternalInput")
    g1p = nc.dram_tensor("g1p", [128, MT], F32, kind="ExternalInput")
    be1p = nc.dram_tensor("be1p", [128, MT], F32, kind="ExternalInput")
    g2p = nc.dram_tensor("g2p", [128, MT], F32, kind="ExternalInput")
    be2p = nc.dram_tensor("be2p", [128, MT], F32, kind="ExternalInput")
    b3t = nc.dram_tensor("b3t", [1, 1], F32, kind="ExternalInput")
    y = nc.dram_tensor("y", [1, B_CORE], F32, kind="ExternalOutput")

    from contextlib import ExitStack
    with tile.TileContext(nc) as tc, ExitStack() as ctx, \
            nc.allow_low_precision(reason="f32r rounding is intentional"):
        const = ctx.enter_context(tc.tile_pool(name="const", bufs=1))
        wstage = ctx.enter_context(tc.tile_pool(name="wstage", bufs=2))
        xin = ctx.enter_context(tc.tile_pool(name="xin", bufs=cfg["xin_bufs"]))
        xr = ctx.enter_context(tc.tile_pool(name="xr", bufs=cfg["xr_bufs"]))
        bigH = ctx.enter_context(tc.tile_pool(name="bigH", bufs=cfg["h_bufs"]))
        bigS = ctx.enter_context(tc.tile_pool(name="bigS", bufs=cfg["sq_bufs"]))
        bigR1 = ctx.enter_context(tc.tile_pool(name="bigR1", bufs=cfg["r1_bufs"]))
        bigR2 = ctx.enter_context(tc.tile_pool(name="bigR2", bufs=cfg["r2_bufs"]))
        small = ctx.enter_context(tc.tile_pool(name="small", bufs=cfg.get("small_bufs", 2)))
        ps_mm = ctx.enter_context(tc.tile_pool(name="ps_mm", bufs=cfg["ps_mm_bufs"], space="PSUM"))
        ps_st = ctx.enter_context(tc.tile_pool(name="ps_st", bufs=cfg.get("ps_st_bufs", 2), space="PSUM"))
        ps_vec = ctx.enter_context(tc.tile_pool(name="ps_vec", bufs=cfg.get("ps_vec_bufs", 2), space="PSUM"))

        w1a_r = const.tile([128, HID], F32R, tag="w1a")
        st = wstage.tile([128, HID], F32, tag="stage")
        nc.sync.dma_start(st[:], w1.ap()[0:128, :])
        nc.vector.tensor_copy(w1a_r[:], st[:])
        w1c_r = const.tile([KC, HID], F32R, tag="w1c")
        stc = wstage.tile([KC, HID], F32, tag="stagec")
        nc.sync.dma_start(stc[:], w1.ap()[128:185, :])
        nc.vector.tensor_copy(w1c_r[:], stc[:])
        L2DT = mybir.dt.float16 if cfg["l2_fp16"] else F32R
        w2r = []
        for k in range(MT):
            stk = wstage.tile([128, HID], F32, tag="stage")
            nc.sync.dma_start(stk[:], w2.ap()[k * 128:(k + 1) * 128, :])
            t = const.tile([128, HID], L2DT, tag=f"w2r{k}")
            nc.vector.tensor_copy(t[:], stk[:])
            w2r.append(t)
        w3p_r = const.tile([128, MT], L2DT, tag="w3p")
        st3 = wstage.tile([128, MT], F32, tag="stages")
        nc.sync.dma_start(st3[:], w3p.ap())
        nc.vector.tensor_copy(w3p_r[:], st3[:])

        def load_small(name, dram):
            t = const.tile([128, MT], F32, tag=name)
            nc.sync.dma_start(t[:], dram.ap())
            return t
        bc1s = load_small("bc1s", bc1p); bc2s = load_small("bc2s", bc2p)
        g1s = load_small("g1s", g1p); be1s = load_small("be1s", be1p)
        g2s = load_small("g2s", g2p); be2s = load_small("be2s", be2p)
        b3s = const.tile([1, 1], F32, tag="b3s")
        nc.sync.dma_start(b3s[:], b3t.ap())
        ones_st = const.tile([128, 1], F32, tag="ones_st")
        nc.vector.memset(ones_st[:], 1.0)
        ones_col = const.tile([128, 1], F32R, tag="ones_col")
        nc.vector.tensor_copy(ones_col[:], ones_st[:])
        ones_rst = const.tile([1, 128], F32, tag="ones_rst")
        nc.vector.memset(ones_rst[:], 1.0)
        ones_row = const.tile([1, 128], F32R, tag="ones_row")
        nc.vector.tensor_copy(ones_row[:], ones_rst[:])
        eps_t = const.tile([1, 1], F32, tag="eps_t")
        nc.vector.memset(eps_t[:], EPS)

        def layer_norm_relu(Hb, g_s, be_s, out_pool, out_tag):
            sqb = bigS.tile([128, MT * CH], F32R, tag="sq")
            if cfg["per_m"]:
                for m in range(MT):
                    sl = slice(m * CH, (m + 1) * CH)
                    nc.vector.tensor_mul(sqb[:, sl], Hb[:, sl], Hb[:, sl])
            else:
                nc.vector.tensor_mul(sqb[:], Hb[:], Hb[:])
            pst = ps_st.tile([1, CH], F32, tag="pst")
            for m in range(MT):
                nc.tensor.matmul(pst[:], ones_col[:],
                                 sqb[:, m * CH:(m + 1) * CH],
                                 start=(m == 0), stop=(m == MT - 1))
            sd = small.tile([1, CH], F32, tag="sd")
            nc.scalar.activation(sd[:], pst[:], AF.Sqrt,
                                 bias=eps_t[:], scale=1.0 / HID)
            rs = small.tile([1, CH], F32R, tag="rs")
            nc.vector.reciprocal(rs[:], sd[:])
            pv = ps_vec.tile([128, CH], F32, tag="pv")
            nc.tensor.matmul(pv[:], ones_row[:], rs[:], start=True, stop=True)
            Rb = out_pool.tile([128, MT * CH], L2DT, tag=out_tag)
            if cfg["per_m"]:
                for m in range(MT):
                    sl = slice(m * CH, (m + 1) * CH)
                    nc.vector.tensor_mul(Hb[:, sl], Hb[:, sl], pv[:])
                    if simple_affine:
                        nc.scalar.activation(Rb[:, sl], Hb[:, sl], AF.Relu)
                    else:
                        nc.scalar.activation(Rb[:, sl], Hb[:, sl], AF.Relu,
                                             bias=be_s[:, m:m + 1],
                                             scale=g_s[:, m:m + 1])
            else:
                h3 = Hb[:].rearrange("p (m n) -> p m n", m=MT)
                pvb = bass.AP(tensor=pv[:].tensor, offset=pv[:].offset,
                              ap=[pv[:].ap[0], [0, MT], pv[:].ap[1]])
                nc.vector.tensor_mul(h3, h3, pvb)
                if simple_affine:
                    nc.scalar.activation(Rb[:], Hb[:], AF.Relu)
                else:
                    for m in range(MT):
                        sl = slice(m * CH, (m + 1) * CH)
                        nc.scalar.activation(Rb[:, sl], Hb[:, sl], AF.Relu,
                                             bias=be_s[:, m:m + 1],
                                             scale=g_s[:, m:m + 1])
            return Rb

        HDT = mybir.dt.float16 if cfg["h_fp16"] else F32

        def chunk_body(c):
            x1 = xin.tile([128, CH], F32, tag="x1")
            nc.sync.dma_start(x1[:], xt.ap()[0:128, c * CH:(c + 1) * CH])
            x2 = xin.tile([KC, CH], F32, tag="x2")
            nc.sync.dma_start(x2[:], xt.ap()[128:185, c * CH:(c + 1) * CH])
            x1r = xr.tile([128, CH], F32R, tag="x1r")
            nc.vector.tensor_copy(x1r[:], x1[:])
            xab = xr.tile([KC, CH], F32, tag="xab")
            nc.vector.tensor_scalar(
                out=xab[:].bitcast(mybir.dt.int32),
                in0=x2[:].bitcast(mybir.dt.int32),
                scalar1=0x7FFFFFFF, scalar2=None, op0=ALU.bitwise_and)
            xln = xr.tile([KC, CH], F32, tag="xln")
            nc.scalar.activation(xln[:], xab[:], AF.Ln, bias=1.0)
            xsg = xr.tile([KC, CH], F32, tag="xsg")
            nc.scalar.activation(xsg[:], x2[:], AF.Sign)
            x2r = xr.tile([KC, CH], F32R, tag="x2r")
            nc.vector.tensor_mul(x2r[:], xsg[:], xln[:])

            H1 = bigH.tile([128, MT * CH], HDT, tag="H")
            for m in range(MT):
                p1 = ps_mm.tile([128, CH], F32, tag="pmm")
                nc.tensor.matmul(p1[:], w1a_r[:, m * 128:(m + 1) * 128],
                                 x1r[:], start=True, stop=False)
                nc.tensor.matmul(p1[:], w1c_r[:, m * 128:(m + 1) * 128],
                                 x2r[:], start=False, stop=True)
                nc.scalar.activation(H1[:, m * CH:(m + 1) * CH], p1[:],
                                     AF.Identity, bias=bc1s[:, m:m + 1])
            R1 = layer_norm_relu(H1, g1s, be1s, bigR1, "R1")

            H2 = bigH.tile([128, MT * CH], HDT, tag="H")
            for m in range(MT):
                p2 = ps_mm.tile([128, CH], F32, tag="pmm")
                for k in range(MT):
                    nc.tensor.matmul(p2[:], w2r[k][:, m * 128:(m + 1) * 128],
                                     R1[:, k * CH:(k + 1) * CH],
                                     start=(k == 0), stop=(k == MT - 1))
                nc.scalar.activation(H2[:, m * CH:(m + 1) * CH], p2[:],
                                     AF.Identity, bias=bc2s[:, m:m + 1])
            R2 = layer_norm_relu(H2, g2s, be2s, bigR2, "R2")

            p3 = ps_st.tile([1, CH], F32, tag="pst")
            for k in range(MT):
                nc.tensor.matmul(p3[:], w3p_r[:, k:k + 1],
                                 R2[:, k * CH:(k + 1) * CH],
                                 start=(k == 0), stop=(k == MT - 1))
            osb = small.tile([1, CH], F32, tag="osb")
            nc.scalar.activation(osb[:], p3[:], AF.Identity, bias=b3s[:])
            nc.sync.dma_start(y.ap()[0:1, c * CH:(c + 1) * CH], osb[:])

        if loop_iters is None:
            for c in range(NCH):
                chunk_body(c)
        else:
            with tc.For_i(0, loop_iters, 1):
                for c in range(NCH):
                    chunk_body(c)
    nc.compile()
    return nc


def _build_xt_legacy(inp):
    XT = np.zeros((KA + KC, inp["op_vec"].shape[0]), np.float32)
    XT[0:185] = _build_xt_raw(inp)
    return XT


def make_legacy_maps(inp):
    W1c, bc1, W2c, bc2 = _fold_weights(inp)
    XT = _build_xt_legacy(inp)
    g1 = np.asarray(inp["g1"], np.float32); be1 = np.asarray(inp["be1"], np.float32)
    g2 = np.asarray(inp["g2"], np.float32); be2 = np.asarray(inp["be2"], np.float32)
    simple_affine = bool(
        np.all(g1 == 1.0) and np.all(g2 == 1.0)
        and np.all(be1 == 0.0) and np.all(be2 == 0.0))
    W3 = np.asarray(inp["W3"], np.float32)
    b3 = np.asarray(inp["b3"], np.float32)
    shared = {
        "w1": W1c, "w2": W2c,
        "w3p": _pack128(W3[:, 0]),
        "bc1p": _pack128(bc1), "bc2p": _pack128(bc2),
        "g1p": _pack128(g1), "be1p": _pack128(be1),
        "g2p": _pack128(g2), "be2p": _pack128(be2),
        "b3t": b3.reshape(1, 1),
    }
    in_maps = []
    for c in range(N_CORES):
        m = dict(shared)
        m["xt"] = np.ascontiguousarray(XT[:, c * B_CORE:(c + 1) * B_CORE])
        in_maps.append(m)
    return in_maps, simple_affine


# ---------------------------------------------------------------- entry point
_CACHE = {}

BEST_CFG = dict(ps_mm_bufs=5, ps_st_bufs=2, ps_p3_bufs=1)
LEGACY_BEST_CFG = dict(per_m=True, h_bufs=2, ps_mm_bufs=4)


def make_in_maps(inputs):
    """Returns (in_maps, mode) with mode = ("fast2", scal),
    ("fast", has_b3) or ("legacy", simple_affine)."""
    inp = {k: np.asarray(v) for k, v in inputs.items()}
    if fast2_ok(inp):
        in_maps, scal = make_fast3_maps(inp)
        return in_maps, ("fast3", scal)
    if fast_path_ok(inp):
        in_maps, has_b3 = make_fast_maps(inp)
        return in_maps, ("fast", has_b3)
    in_maps, simple_affine = make_legacy_maps(inp)
    return in_maps, ("legacy", simple_affine)


def build_for_mode(mode, loop_iters=None, cfg=None):
    kind, flag = mode
    if kind == "fast4":
        return build_fast4(flag, loop_iters=loop_iters, cfg=cfg)
    if kind == "fast3":
        return build_fast3(flag, loop_iters=loop_iters, cfg=cfg)
    if kind == "fast2":
        return build_fast2(flag, loop_iters=loop_iters, cfg=cfg)
    if kind == "fast":
        return build_fast(flag, loop_iters=loop_iters,
                          cfg=cfg if cfg is not None else BEST_CFG)
    return build_program(flag, loop_iters=loop_iters,
                         cfg=cfg if cfg is not None else LEGACY_BEST_CFG)


def _get_program(mode):
    key = ("prog", mode)
    if key not in _CACHE:
        _CACHE[key] = build_for_mode(mode)
    return _CACHE[key]


def kernel(**inputs) -> np.ndarray:
    in_maps, mode = make_in_maps(inputs)
    nc = _get_program(mode)
    res = run_bass_kernel_spmd(nc, in_maps, core_ids=list(range(N_CORES)))
    y = np.concatenate([r["y"][0] for r in res.results])
    return y.reshape(B, 1).astype(np.float32)


if __name__ == "__main__":
    import jax
    import reference
    cpu = jax.devices("cpu")[0]
    with jax.default_device(cpu):
        inp = reference.setup_inputs()
        ref = np.asarray(reference.reference(**inp))
    out = kernel(**{k: np.asarray(v) for k, v in inp.items()})
    err = np.abs(out - ref)
    scale = np.abs(ref).max()
    print("max_abs", err.max(), "rel(vs scale)", err.max() / scale,
          "mean_rel", (err / (np.abs(ref) + 1e-6)).mean())

